# revision 16
# baseline (speedup 1.0000x reference)
"""Trainium2 Bass kernel for nn_AtrousAttentionBlock (16 dilated attention branches + smooth conv).

Sharding: 8 cores = (image n in [0,4)) x (vertical half v in [0,2)); core 2n+v computes
output rows [64v, 64v+64) of image n from a zero-padded 100-row x shard (17-row halos).
On-chip layout is channel-major bf16 (f32 PSUM accumulation):
  phase 1: LN_in (pixel-major stats) + PE transpose -> xn channel-major
  phase 2: 1x1 "dwn" conv, all 16 branches packed as 128 partitions (16br x 8ch)
  phase 3: per-branch dilation shifts materialized via DRAM im2col round trip
  phase 4: k/q/v 3x3 dilated convs as block-diagonal-weight matmuls (9 taps accumulated)
  phase 5: spatial-softmax partial sums + pairwise AllReduce (image halves)
  phase 6: attn*v + LN_up (stats via block-ones matmul, broadcast fused)
  phase 7: 3x3 smooth conv 384->256 (27 accumulating K-chunk matmuls per pixel tile)
  phase 8: LN_out in pixel-major after PE transpose, DMA out in NHWC
"""
import numpy as np
import ml_dtypes

import concourse.bass as bass
import concourse.bacc as bacc
import concourse.tile as tile
import concourse.mybir as mybir
from concourse.bass_utils import run_bass_kernel_spmd

N, H, W, C = 4, 128, 128, 256
ND, DF = 16, 8
CAT = C + ND * DF
EPS = 1e-3
N_CORES = 8

HALO = 17        # x halo rows per side (16 for dilation-16 taps + 1 for smooth conv)
SR = 100         # stored shard rows (98 used, padded to 4-row groups)
XT_W = 130       # padded width of channel-major xn/y (1 zero col per side)
XT_R = 68        # xn rows kept for the smooth conv (shard rows 16..84)
DX_R = 104       # dx rows ([0,98) real + zero tail so all 9x8-row kqv chunks stay in range)
DX_W = 160       # dx padded width (16 zero cols per side)
KR = 66          # kqv rows used: image rows 64v-1 .. 64v+65

F32 = mybir.dt.float32
BF16 = mybir.dt.bfloat16
AF = mybir.ActivationFunctionType
ALU = mybir.AluOpType

_CACHE = {}


# --------------------------------------------------------------------------
# host-side preparation
# --------------------------------------------------------------------------
def _prep_weights(inp):
    """Fold LN affines into conv weights; build PE-layout weight matrices."""
    for k in ("ln_in_b", "ln_up_b", "ln_out_b"):
        assert np.all(np.asarray(inp[k]) == 0), f"{k} != 0 unsupported"
    assert np.all(np.asarray(inp["ln_out_g"]) == 1), "ln_out_g != 1 unsupported"
    g_in = np.asarray(inp["ln_in_g"], np.float64)
    g_up = np.asarray(inp["ln_up_g"], np.float64)

    W_dwn = np.zeros((C, 128), np.float64)
    for i in range(ND):
        W_dwn[:, 8 * i:8 * i + 8] = inp["dwn_w"][i, 0, 0] * g_in[:, None]

    def blockdiag(w):  # [ND,3,3,DF,DF] -> [9,128,128]
        out = np.zeros((9, 128, 128), np.float64)
        for t in range(9):
            ky, kx = t // 3, t % 3
            for i in range(ND):
                out[t, 8 * i:8 * i + 8, 8 * i:8 * i + 8] = w[i, ky, kx]
        return out

    Ws = np.asarray(inp["smooth_w"], np.float64).copy()
    Ws[:, :, :C, :] *= g_in[None, None, :, None]
    for i in range(ND):
        Ws[:, :, C + 8 * i:C + 8 * i + 8, :] *= g_up[None, None, :, None]

    Gm = np.zeros((128, 128), np.float64)   # per-branch mean matrix (ones8x8 / 8)
    for i in range(ND):
        Gm[8 * i:8 * i + 8, 8 * i:8 * i + 8] = 1.0 / 8.0

    bf = lambda a: np.ascontiguousarray(np.asarray(a, np.float32)).astype(ml_dtypes.bfloat16)
    f32c = lambda a: np.ascontiguousarray(np.asarray(a, np.float32))
    biases = np.stack([
        np.asarray(inp["dwn_b"]).reshape(128),
        np.asarray(inp["k_b"]).reshape(128),
        np.asarray(inp["q_b"]).reshape(128),
        np.asarray(inp["v_b"]).reshape(128),
        np.asarray(inp["smooth_b"])[0:128],
        np.asarray(inp["smooth_b"])[128:256],
    ], axis=1)
    return dict(
        wdwn=bf(W_dwn),
        wk=bf(blockdiag(inp["k_w"])), wq=bf(blockdiag(inp["q_w"])), wv=bf(blockdiag(inp["v_w"])),
        ws=bf(Ws.reshape(9, 3, 128, 256)),
        biases=f32c(biases),
        gm=bf(Gm),
        ident=bf(np.eye(128)),
    )


def _shard_x(x):
    shards = []
    for c in range(N_CORES):
        n, v = c // 2, c % 2
        sh = np.zeros((SR, W, C), np.float32)
        lo, hi = 64 * v - HALO, 64 * v + 81
        slo, shi = max(0, lo), min(H, hi)
        sh[slo - lo:shi - lo] = x[n, slo:shi]
        shards.append(np.ascontiguousarray(sh))
    return shards


# --------------------------------------------------------------------------
# device kernel
# --------------------------------------------------------------------------
def build_kernel():
    nc = bacc.Bacc("TRN2", target_bir_lowering=False, debug=False, num_devices=N_CORES)

    x_in = nc.dram_tensor("x", [SR, W, C], F32, kind="ExternalInput")
    wdwn_in = nc.dram_tensor("wdwn", [C, 128], BF16, kind="ExternalInput")
    wk_in = nc.dram_tensor("wk", [9, 128, 128], BF16, kind="ExternalInput")
    wq_in = nc.dram_tensor("wq", [9, 128, 128], BF16, kind="ExternalInput")
    wv_in = nc.dram_tensor("wv", [9, 128, 128], BF16, kind="ExternalInput")
    ws_in = nc.dram_tensor("ws", [9, 3, 128, 256], BF16, kind="ExternalInput")
    bias_in = nc.dram_tensor("biases", [128, 6], F32, kind="ExternalInput")
    gm_in = nc.dram_tensor("gm", [128, 128], BF16, kind="ExternalInput")
    id_in = nc.dram_tensor("ident", [128, 128], BF16, kind="ExternalInput")
    out = nc.dram_tensor("out", [64, W, C], F32, kind="ExternalOutput")

    with tile.TileContext(nc) as tc:
        with (
            tc.tile_pool(name="const", bufs=1) as cp,
            tc.tile_pool(name="big", bufs=1) as bp,
            tc.tile_pool(name="work", bufs=2) as wk,
            tc.tile_pool(name="wk1", bufs=1) as wk1,
            tc.tile_pool(name="sht", bufs=2) as shp,
            tc.tile_pool(name="st", bufs=3) as st,
            tc.tile_pool(name="ps", bufs=2, space="PSUM") as ps,
            tc.tile_pool(name="acc", bufs=6, space="PSUM") as acc,
            tc.tile_pool(name="dram", bufs=1, space="DRAM") as dram,
        ):
            # ---- constants ----
            Wdwn = cp.tile([128, 2, 128], BF16, tag="wdwn")
            nc.sync.dma_start(Wdwn[:], wdwn_in.ap().rearrange("(c p) m -> p c m", p=128))
            Wk = cp.tile([128, 9, 128], BF16, tag="wk")
            Wq = cp.tile([128, 9, 128], BF16, tag="wq")
            Wv = cp.tile([128, 9, 128], BF16, tag="wv")
            Ws = cp.tile([128, 9, 3, 256], BF16, tag="ws")
            nc.sync.dma_start(Wk[:], wk_in.ap().rearrange("t p m -> p t m"))
            nc.sync.dma_start(Wq[:], wq_in.ap().rearrange("t p m -> p t m"))
            nc.sync.dma_start(Wv[:], wv_in.ap().rearrange("t p m -> p t m"))
            nc.sync.dma_start(Ws[:], ws_in.ap().rearrange("t k p m -> p t k m"))
            Bias = cp.tile([128, 6], F32, tag="bias")
            nc.sync.dma_start(Bias[:], bias_in.ap())
            Bdwn = Bias[:, 0:1]
            Bk = Bias[:, 1:2]
            Bq = Bias[:, 2:3]
            Bv = Bias[:, 3:4]
            Gm = cp.tile([128, 128], BF16, tag="gm")
            Ident = cp.tile([128, 128], BF16, tag="id")
            nc.sync.dma_start(Gm[:], gm_in.ap())
            nc.sync.dma_start(Ident[:], id_in.ap())

            # ---- persistent activations ----
            xt = bp.tile([128, 2, XT_R, XT_W], BF16, tag="xt")    # xn shard rows 16..84
            dx = bp.tile([128, DX_R, DX_W], BF16, tag="dx")
            ekq = bp.tile([128, KR, 128], BF16, tag="ekq")
            vbuf = bp.tile([128, KR, 128], BF16, tag="vbuf")
            ybuf = bp.tile([128, KR, XT_W], BF16, tag="ybuf")

            nc.vector.memset(xt[:, :, :, 0:1], 0.0)
            nc.vector.memset(xt[:, :, :, XT_W - 1:XT_W], 0.0)
            nc.vector.memset(dx[:, :, 0:16], 0.0)
            nc.vector.memset(dx[:, :, DX_W - 16:DX_W], 0.0)
            nc.vector.memset(dx[:, 98:DX_R, 16:DX_W - 16], 0.0)
            nc.vector.memset(ybuf[:, :, 0:1], 0.0)
            nc.vector.memset(ybuf[:, :, XT_W - 1:XT_W], 0.0)

            # per-(tap,branch) flat-offset copies: dst[32-s+f] = dx_flat[rs0*160+f]
            # read window [16:144) of each 160-pitch row then sees dx[rs0+j, c+s].
            sh_dram = dram.tile([9, 128, 32 + 72 * 160 + 32], BF16, tag="shd")
            cc_in = dram.tile([128, 1], F32, tag="ccin")
            cc_out = dram.tile([128, 1], F32, tag="ccout")

            # ============ phase 1+2: LN_in + transpose + dwn conv ============
            IM2COL_SCHED = {}
            for ky in range(3):
                for i in range(ND):
                    d = i + 1
                    rs0 = 16 + (ky - 1) * d
                    g_ready = max(0, (min(rs0 + 72, 98) + 3) // 4 - 1)
                    IM2COL_SCHED.setdefault(g_ready, []).append((ky, i, rs0, d))
            SH_PS = 128 * 11584  # sh_dram plane stride (elements)
            for g in range(SR // 4):
                r0 = 4 * g
                xg = wk.tile([128, 4, 256], F32, tag="xg")
                nc.gpsimd.dma_start(xg[:], x_in.ap()[r0:r0 + 4].rearrange("r w c -> w r c"))
                bst = st.tile([128, 4, 6], F32, tag="bst")
                for j in range(4):
                    nc.vector.bn_stats(bst[:, j, :], xg[:, j, :])
                mv = st.tile([128, 4, 2], F32, tag="mv")
                for j in range(4):
                    nc.vector.bn_aggr(mv[:, j, :], bst[:, j, :])
                sd = st.tile([128, 4], F32, tag="sd")
                nc.vector.tensor_scalar_add(sd[:], mv[:, :, 1], EPS)
                r = st.tile([128, 4], F32, tag="r")
                nc.scalar.activation(r[:], sd[:], AF.Abs_reciprocal_sqrt)

                nmr = st.tile([128, 4], F32, tag="nmr")
                nc.vector.tensor_tensor(nmr[:], mv[:, :, 0], r[:], op=ALU.mult)
                nc.vector.tensor_scalar_mul(nmr[:], nmr[:], -1.0)
                xnb = wk.tile([128, 4, 256], BF16, tag="xnb")
                for j in range(4):
                    nc.scalar.activation(xnb[:, j, :], xg[:, j, :], AF.Identity,
                                         bias=nmr[:, j:j + 1], scale=r[:, j:j + 1])
                # transpose to channel-major
                in_cat = 16 <= r0 < 84
                if in_cat:
                    dst = xt
                    roff = r0 - 16
                else:
                    dst = wk1.tile([128, 2, 4, XT_W], BF16, tag="xnt")
                    roff = 0
                for j in range(4):
                    tp = ps.tile([128, 256], BF16, tag="mm")
                    for ch in range(2):
                        nc.tensor.transpose(tp[:, 128 * ch:128 * ch + 128],
                                            xnb[:, j, 128 * ch:128 * ch + 128], Ident[:])
                    nc.vector.tensor_copy(
                        dst[:, 0:2, roff + j, 1:129],
                        tp[:].rearrange("p (c b) -> p c b", c=2))
                # dwn conv on these 4 rows
                pd = ps.tile([128, 512], F32, tag="mm")
                pdv = pd[:].rearrange("p (a b) -> p a b", a=4)
                for ch in range(2):
                    nc.tensor.matmul(pdv, Wdwn[:, ch, :],
                                     dst[:, ch, roff:roff + 4, 1:129],
                                     start=(ch == 0), stop=(ch == 1))
                nc.scalar.activation(dx[:, r0:r0 + 4, 16:144], pdv, AF.Relu, bias=Bdwn)
                # merged im2col copies (one DMA per (ky,branch) covers kx=0,1,2 via
                # broadcast src + arithmetic-stride dst; flat shift trick: the kx
                # column shift becomes a flat dst offset, row wrap lands in zero pads)
                for (ky, i, rs0, d) in IM2COL_SCHED.get(g, []):
                    srcf = dx[8 * i:8 * i + 8, rs0:rs0 + 72, :]
                    srcf = srcf.rearrange("p a b -> p (a b)")
                    srcf = srcf.rearrange("p (u f) -> p u f", u=1).broadcast_to([8, 3, 72 * 160])
                    c0 = 3 * ky * SH_PS + 8 * i * 11584 + 32 + d
                    dstf = bass.AP(sh_dram[:].tensor, c0,
                                   [[SH_PS - d, 3], [11584, 8], [1, 72 * 160]])
                    eng = nc.sync if (ky * ND + i) % 2 == 0 else nc.scalar
                    eng.dma_start(dstf, srcf)

            # (im2col writes were interleaved into the phase-1/2 loop above)

            # ============ phase 4: k/q/v convs + exp(kq) ============
            PARTIALS = []
            psums_t = st.tile([128, 20], F32, tag="psums", name="psums_t")
            for c9 in range(9):
                j0 = 8 * c9
                grps = [g for g in range(2) if j0 + 4 * g < KR]
                pk, pq, pv = {}, {}, {}
                for g in grps:
                    pk[g] = acc.tile([128, 512], F32, tag="acc", name=f"pk{c9}_{g}")
                    pq[g] = acc.tile([128, 512], F32, tag="acc", name=f"pq{c9}_{g}")
                    pv[g] = acc.tile([128, 512], F32, tag="acc", name=f"pv{c9}_{g}")
                for tg in range(3):
                    sht = shp.tile([128, 3, 8, 160], BF16, tag="sht")
                    eng = nc.sync if tg % 2 == 0 else nc.scalar
                    eng.dma_start(
                        sht[:].rearrange("p t a b -> p t (a b)"),
                        sh_dram[3 * tg:3 * tg + 3, :,
                                32 + j0 * 160:32 + (j0 + 8) * 160].rearrange(
                                    "t p f -> p t f"))
                    for tt in range(3):
                        t = 3 * tg + tt
                        for grp in grps:
                            rhs = sht[:, tt, 4 * grp:4 * grp + 4, 16:144]
                            nc.tensor.matmul(pk[grp][:], Wk[:, t, :], rhs,
                                             start=(t == 0), stop=(t == 8))
                            nc.tensor.matmul(pq[grp][:], Wq[:, t, :], rhs,
                                             start=(t == 0), stop=(t == 8))
                            nc.tensor.matmul(pv[grp][:], Wv[:, t, :], rhs,
                                             start=(t == 0), stop=(t == 8))
                for grp in grps:
                    jb = j0 + 4 * grp
                    nrows = min(4, KR - jb)
                    nn = 128 * nrows
                    kev = wk.tile([128, 4, 128], BF16, tag="kev")
                    qev = wk.tile([128, 4, 128], BF16, tag="qev")
                    nc.scalar.activation(kev[:, 0:nrows, :], pk[grp][:, 0:nn], AF.Relu,
                                         bias=Bk)
                    nc.scalar.activation(qev[:, 0:nrows, :], pq[grp][:, 0:nn], AF.Relu,
                                         bias=Bq)
                    nc.scalar.activation(vbuf[:, jb:jb + nrows, :], pv[grp][:, 0:nn],
                                         AF.Relu, bias=Bv)
                    kqf = wk.tile([128, 4, 128], F32, tag="kqf")
                    nc.vector.tensor_tensor(kqf[:, 0:nrows, :], kev[:, 0:nrows, :],
                                            qev[:, 0:nrows, :], op=ALU.mult)
                    nc.scalar.activation(ekq[:, jb:jb + nrows, :], kqf[:, 0:nrows, :],
                                         AF.Exp)
                    # partial softmax sum over the fresh rows restricted to [1, 65)
                    lo = max(jb, 1)
                    hi = min(jb + nrows, 65)
                    if lo < hi:
                        pidx = len(PARTIALS)
                        psl = psums_t[:, pidx:pidx + 1]
                        nc.vector.tensor_reduce(psl, ekq[:, lo:hi, :],
                                                axis=mybir.AxisListType.XY, op=ALU.add)
                        PARTIALS.append(pidx)

            # ============ phase 5: softmax sums + pairwise AllReduce ============
            sums = st.tile([128, 1], F32, tag="sums")
            nc.vector.tensor_reduce(sums[:], psums_t[:, 0:len(PARTIALS)],
                                    axis=mybir.AxisListType.X, op=ALU.add)
            nc.sync.dma_start(cc_in[:], sums[:])
            nc.gpsimd.collective_compute(
                "AllReduce", ALU.add,
                replica_groups=[[0, 1], [2, 3], [4, 5], [6, 7]],
                ins=[cc_in.opt()], outs=[cc_out.opt()],
            )
            tsum = st.tile([128, 1], F32, tag="tsum")
            nc.sync.dma_start(tsum[:], cc_out[:])
            rs = st.tile([128, 1], F32, tag="rs")
            nc.vector.reciprocal(rs[:], tsum[:])

            # ============ phase 6: attn*v + LN_up -> ybuf ============
            for c22 in range(22):
                j0 = 3 * c22
                avf = wk.tile([128, 3, 128], F32, tag="avf")
                nc.vector.tensor_tensor(avf[:], ekq[:, j0:j0 + 3, :],
                                        vbuf[:, j0:j0 + 3, :], op=ALU.mult)
                avb = wk.tile([128, 3, 128], BF16, tag="avb")
                nc.scalar.activation(avb[:], avf[:], AF.Identity, scale=rs[:])
                sqb = wk.tile([128, 3, 128], BF16, tag="sqb")
                nc.scalar.activation(sqb[:], avb[:], AF.Square)
                pm = acc.tile([128, 384], F32, tag="acc")
                pq2 = acc.tile([128, 384], F32, tag="acc")
                nc.tensor.matmul(pm[:], Gm[:], avb[:].rearrange("p a b -> p (a b)"),
                                 start=True, stop=True)
                nc.tensor.matmul(pq2[:], Gm[:], sqb[:].rearrange("p a b -> p (a b)"),
                                 start=True, stop=True)
                msb = wk1.tile([128, 384], F32, tag="msb")
                nc.scalar.copy(msb[:], pm[:])
                m2 = wk1.tile([128, 384], F32, tag="m2")
                nc.scalar.activation(m2[:], msb[:], AF.Square)
                varu = wk1.tile([128, 384], F32, tag="varu")
                nc.vector.tensor_tensor(varu[:], pq2[:], m2[:], op=ALU.subtract)
                nc.vector.tensor_scalar_add(varu[:], varu[:], EPS)
                ru = wk1.tile([128, 384], F32, tag="ru")
                nc.scalar.activation(ru[:], varu[:], AF.Abs_reciprocal_sqrt)
                yt = wk1.tile([128, 384], F32, tag="yt")
                nc.vector.tensor_tensor(yt[:], avb[:].rearrange("p a b -> p (a b)"),
                                        msb[:], op=ALU.subtract)
                nc.vector.tensor_tensor(
                    ybuf[:, j0:j0 + 3, 1:129],
                    yt[:].rearrange("p (a b) -> p a b", a=3),
                    ru[:].rearrange("p (a b) -> p a b", a=3), op=ALU.mult)

            # ============ phase 7+8: smooth conv + LN_out ============
            for pc in range(16):
                r0 = 4 * pc
                op_t = wk.tile([128, 2, 4, 128], BF16, tag="opt")
                for mc in range(2):
                    psm = ps.tile([128, 512], F32, tag="mm")
                    first = True
                    for t in range(9):
                        ky, kx = t // 3, t % 3
                        for kc in range(3):
                            if kc < 2:
                                rhs = xt[:, kc, r0 + ky:r0 + ky + 4, kx:kx + 128]
                            else:
                                rhs = ybuf[:, r0 + ky:r0 + ky + 4, kx:kx + 128]
                            nc.tensor.matmul(
                                psm[:], Ws[:, t, kc, 128 * mc:128 * mc + 128], rhs,
                                start=first, stop=(t == 8 and kc == 2))
                            first = False
                    nc.scalar.activation(op_t[:, mc, :, :],
                                         psm[:].rearrange("p (a b) -> p a b", a=4),
                                         AF.Relu, bias=Bias[:, 4 + mc:5 + mc])
                # transpose to pixel-major
                on_t = wk.tile([128, 4, 256], BF16, tag="ont")
                for j in range(4):
                    for mc in range(2):
                        tp = ps.tile([128, 256], BF16, tag="mm")
                        nc.tensor.transpose(tp[:, 0:128], op_t[:, mc, j, :], Ident[:])
                        nc.vector.tensor_copy(on_t[:, j, 128 * mc:128 * mc + 128],
                                              tp[:, 0:128])
                # LN_out (over 256 channels, free dim now)
                bsto = st.tile([128, 4, 6], F32, tag="bsto")
                for j in range(4):
                    nc.vector.bn_stats(bsto[:, j, :], on_t[:, j, :])
                mvo = st.tile([128, 4, 2], F32, tag="mvo")
                for j in range(4):
                    nc.vector.bn_aggr(mvo[:, j, :], bsto[:, j, :])
                sdo = st.tile([128, 4], F32, tag="sdo")
                nc.vector.tensor_scalar_add(sdo[:], mvo[:, :, 1], EPS)
                ro = st.tile([128, 4], F32, tag="ro")
                nc.scalar.activation(ro[:], sdo[:], AF.Abs_reciprocal_sqrt)
                orow = wk.tile([128, 4, 256], F32, tag="orow")
                for j in range(4):
                    nc.vector.tensor_scalar(orow[:, j, :], on_t[:, j, :],
                                            mvo[:, j, 0:1], ro[:, j:j + 1],
                                            op0=ALU.subtract, op1=ALU.mult)
                nc.gpsimd.dma_start(out.ap()[r0:r0 + 4].rearrange("r w c -> w r c"),
                                     orow[:])

    nc.compile()
    return nc


# --------------------------------------------------------------------------
# public entry point
# --------------------------------------------------------------------------
def kernel(**inputs):
    if "nc" not in _CACHE:
        _CACHE["nc"] = build_kernel()
    nc = _CACHE["nc"]

    wts = _prep_weights(inputs)
    shards = _shard_x(np.asarray(inputs["x"], np.float32))
    in_maps = []
    for c in range(N_CORES):
        m = {"x": shards[c]}
        m.update(wts)
        in_maps.append(m)
    res = run_bass_kernel_spmd(nc, in_maps, core_ids=list(range(N_CORES)))
    full = np.empty((N, H, W, C), np.float32)
    for c in range(N_CORES):
        n, v = c // 2, c % 2
        full[n, 64 * v:64 * v + 64] = res.results[c]["out"]
    return full


# revision 17
# speedup vs baseline: 2.2020x; 2.2020x over previous
"""Trainium2 Bass kernel for nn_AtrousAttentionBlock (16 dilated attention branches + smooth conv).

Sharding: 8 cores = (image n in [0,4)) x (vertical half v in [0,2)); core 2n+v computes
output rows [64v, 64v+64) of image n from a zero-padded 100-row x shard (17-row halos).
On-chip layout is channel-major bf16 (f32 PSUM accumulation):
  phase 1: LN_in (pixel-major stats) + PE transpose -> xn channel-major
  phase 2: 1x1 "dwn" conv, all 16 branches packed as 128 partitions (16br x 8ch)
  phase 3: per-branch dilation shifts materialized via DRAM im2col round trip
  phase 4: k/q/v 3x3 dilated convs as block-diagonal-weight matmuls (9 taps accumulated)
  phase 5: spatial-softmax partial sums + pairwise AllReduce (image halves)
  phase 6: attn*v + LN_up (stats via block-ones matmul, broadcast fused)
  phase 7: 3x3 smooth conv 384->256 (27 accumulating K-chunk matmuls per pixel tile)
  phase 8: LN_out in pixel-major after PE transpose, DMA out in NHWC
"""
import numpy as np
import ml_dtypes

import concourse.bass as bass
import concourse.bacc as bacc
import concourse.tile as tile
import concourse.mybir as mybir
from concourse.bass_utils import run_bass_kernel_spmd

N, H, W, C = 4, 128, 128, 256
ND, DF = 16, 8
CAT = C + ND * DF
EPS = 1e-3
N_CORES = 8

HALO = 17        # x halo rows per side (16 for dilation-16 taps + 1 for smooth conv)
SR = 100         # stored shard rows (98 used, padded to 4-row groups)
XT_W = 130       # padded width of channel-major xn/y (1 zero col per side)
XT_R = 68        # xn rows kept for the smooth conv (shard rows 16..84)
DX_R = 104       # dx rows ([0,98) real + zero tail so all 9x8-row kqv chunks stay in range)
DX_W = 160       # dx padded width (16 zero cols per side)
KR = 66          # kqv rows used: image rows 64v-1 .. 64v+65

F32 = mybir.dt.float32
BF16 = mybir.dt.bfloat16
AF = mybir.ActivationFunctionType
ALU = mybir.AluOpType

_CACHE = {}


# --------------------------------------------------------------------------
# host-side preparation
# --------------------------------------------------------------------------
def _prep_weights(inp):
    """Fold LN affines into conv weights; build PE-layout weight matrices."""
    for k in ("ln_in_b", "ln_up_b", "ln_out_b"):
        assert np.all(np.asarray(inp[k]) == 0), f"{k} != 0 unsupported"
    assert np.all(np.asarray(inp["ln_out_g"]) == 1), "ln_out_g != 1 unsupported"
    g_in = np.asarray(inp["ln_in_g"], np.float64)
    g_up = np.asarray(inp["ln_up_g"], np.float64)

    W_dwn = np.zeros((C, 128), np.float64)
    for i in range(ND):
        W_dwn[:, 8 * i:8 * i + 8] = inp["dwn_w"][i, 0, 0] * g_in[:, None]

    def blockdiag(w):  # [ND,3,3,DF,DF] -> [9,128,128]
        out = np.zeros((9, 128, 128), np.float64)
        for t in range(9):
            ky, kx = t // 3, t % 3
            for i in range(ND):
                out[t, 8 * i:8 * i + 8, 8 * i:8 * i + 8] = w[i, ky, kx]
        return out

    Ws = np.asarray(inp["smooth_w"], np.float64).copy()
    Ws[:, :, :C, :] *= g_in[None, None, :, None]
    for i in range(ND):
        Ws[:, :, C + 8 * i:C + 8 * i + 8, :] *= g_up[None, None, :, None]

    Gm = np.zeros((128, 128), np.float64)   # per-branch mean matrix (ones8x8 / 8)
    for i in range(ND):
        Gm[8 * i:8 * i + 8, 8 * i:8 * i + 8] = 1.0 / 8.0

    bf = lambda a: np.ascontiguousarray(np.asarray(a, np.float32)).astype(ml_dtypes.bfloat16)
    f32c = lambda a: np.ascontiguousarray(np.asarray(a, np.float32))
    biases = np.stack([
        np.asarray(inp["dwn_b"]).reshape(128),
        np.asarray(inp["k_b"]).reshape(128),
        np.asarray(inp["q_b"]).reshape(128),
        np.asarray(inp["v_b"]).reshape(128),
        np.asarray(inp["smooth_b"])[0:128],
        np.asarray(inp["smooth_b"])[128:256],
    ], axis=1)
    return dict(
        wdwn=bf(W_dwn),
        wk=bf(blockdiag(inp["k_w"])), wq=bf(blockdiag(inp["q_w"])), wv=bf(blockdiag(inp["v_w"])),
        ws=bf(Ws.reshape(9, 3, 128, 256)),
        biases=f32c(biases),
        gm=bf(Gm),
        ident=bf(np.eye(128)),
    )


def _shard_x(x):
    shards = []
    for c in range(N_CORES):
        n, v = c // 2, c % 2
        sh = np.zeros((SR, W, C), np.float32)
        lo, hi = 64 * v - HALO, 64 * v + 81
        slo, shi = max(0, lo), min(H, hi)
        sh[slo - lo:shi - lo] = x[n, slo:shi]
        shards.append(np.ascontiguousarray(sh))
    return shards


# --------------------------------------------------------------------------
# device kernel
# --------------------------------------------------------------------------
def build_kernel():
    nc = bacc.Bacc("TRN2", target_bir_lowering=False, debug=False, num_devices=N_CORES)

    x_in = nc.dram_tensor("x", [SR, W, C], F32, kind="ExternalInput")
    wdwn_in = nc.dram_tensor("wdwn", [C, 128], BF16, kind="ExternalInput")
    wk_in = nc.dram_tensor("wk", [9, 128, 128], BF16, kind="ExternalInput")
    wq_in = nc.dram_tensor("wq", [9, 128, 128], BF16, kind="ExternalInput")
    wv_in = nc.dram_tensor("wv", [9, 128, 128], BF16, kind="ExternalInput")
    ws_in = nc.dram_tensor("ws", [9, 3, 128, 256], BF16, kind="ExternalInput")
    bias_in = nc.dram_tensor("biases", [128, 6], F32, kind="ExternalInput")
    gm_in = nc.dram_tensor("gm", [128, 128], BF16, kind="ExternalInput")
    id_in = nc.dram_tensor("ident", [128, 128], BF16, kind="ExternalInput")
    out = nc.dram_tensor("out", [64, W, C], F32, kind="ExternalOutput")

    with tile.TileContext(nc) as tc:
        with (
            tc.tile_pool(name="const", bufs=1) as cp,
            tc.tile_pool(name="big", bufs=1) as bp,
            tc.tile_pool(name="work", bufs=2) as wk,
            tc.tile_pool(name="wk1", bufs=1) as wk1,
            tc.tile_pool(name="sht", bufs=2) as shp,
            tc.tile_pool(name="st", bufs=3) as st,
            tc.tile_pool(name="ps", bufs=2, space="PSUM") as ps,
            tc.tile_pool(name="acc", bufs=6, space="PSUM") as acc,
            tc.tile_pool(name="dram", bufs=1, space="DRAM") as dram,
        ):
            # ---- constants ----
            Wdwn = cp.tile([128, 2, 128], BF16, tag="wdwn")
            nc.sync.dma_start(Wdwn[:], wdwn_in.ap().rearrange("(c p) m -> p c m", p=128))
            Wk = cp.tile([128, 9, 128], BF16, tag="wk")
            Wq = cp.tile([128, 9, 128], BF16, tag="wq")
            Wv = cp.tile([128, 9, 128], BF16, tag="wv")
            Ws = cp.tile([128, 9, 3, 256], BF16, tag="ws")
            nc.sync.dma_start(Wk[:], wk_in.ap().rearrange("t p m -> p t m"))
            nc.sync.dma_start(Wq[:], wq_in.ap().rearrange("t p m -> p t m"))
            nc.sync.dma_start(Wv[:], wv_in.ap().rearrange("t p m -> p t m"))
            nc.sync.dma_start(Ws[:], ws_in.ap().rearrange("t k p m -> p t k m"))
            Bias = cp.tile([128, 6], F32, tag="bias")
            nc.sync.dma_start(Bias[:], bias_in.ap())
            Bdwn = Bias[:, 0:1]
            Bk = Bias[:, 1:2]
            Bq = Bias[:, 2:3]
            Bv = Bias[:, 3:4]
            Gm = cp.tile([128, 128], BF16, tag="gm")
            Ident = cp.tile([128, 128], BF16, tag="id")
            nc.sync.dma_start(Gm[:], gm_in.ap())
            nc.sync.dma_start(Ident[:], id_in.ap())

            # ---- persistent activations ----
            xt = bp.tile([128, 2, XT_R, XT_W], BF16, tag="xt")    # xn shard rows 16..84
            dx = bp.tile([128, DX_R, DX_W], BF16, tag="dx")
            ekq = bp.tile([128, KR, 128], BF16, tag="ekq")
            vbuf = bp.tile([128, KR, 128], BF16, tag="vbuf")
            ybuf = bp.tile([128, KR, XT_W], BF16, tag="ybuf")

            nc.vector.memset(xt[:, :, :, 0:1], 0.0)
            nc.vector.memset(xt[:, :, :, XT_W - 1:XT_W], 0.0)
            nc.vector.memset(dx[:, :, 0:16], 0.0)
            nc.vector.memset(dx[:, :, DX_W - 16:DX_W], 0.0)
            nc.vector.memset(dx[:, 98:DX_R, 16:DX_W - 16], 0.0)
            nc.vector.memset(ybuf[:, :, 0:1], 0.0)
            nc.vector.memset(ybuf[:, :, XT_W - 1:XT_W], 0.0)

            # per-(tap,branch) flat-offset copies: dst[32-s+f] = dx_flat[rs0*160+f]
            # read window [16:144) of each 160-pitch row then sees dx[rs0+j, c+s].
            sh_dram = dram.tile([9, 128, 32 + 72 * 160 + 32], BF16, tag="shd")
            cc_in = dram.tile([128, 1], F32, tag="ccin")
            cc_out = dram.tile([128, 1], F32, tag="ccout")

            # ============ phase 1+2: LN_in + transpose + dwn conv ============
            IM2COL_SCHED = {}
            for t in range(9):
                ky, kx = t // 3, t % 3
                for i in range(ND):
                    d = i + 1
                    rs0 = 16 + (ky - 1) * d
                    s = (kx - 1) * d
                    g_ready = max(0, (min(rs0 + 72, 98) + 3) // 4 - 1)
                    IM2COL_SCHED.setdefault(g_ready, []).append((t, i, rs0, s))
            for g in range(SR // 4):
                r0 = 4 * g
                xg = wk.tile([128, 4, 256], F32, tag="xg")
                nc.gpsimd.dma_start(xg[:], x_in.ap()[r0:r0 + 4].rearrange("r w c -> w r c"))
                bst = st.tile([128, 4, 6], F32, tag="bst")
                for j in range(4):
                    nc.vector.bn_stats(bst[:, j, :], xg[:, j, :])
                mv = st.tile([128, 4, 2], F32, tag="mv")
                for j in range(4):
                    nc.vector.bn_aggr(mv[:, j, :], bst[:, j, :])
                sd = st.tile([128, 4], F32, tag="sd")
                nc.vector.tensor_scalar_add(sd[:], mv[:, :, 1], EPS)
                r = st.tile([128, 4], F32, tag="r")
                nc.scalar.activation(r[:], sd[:], AF.Abs_reciprocal_sqrt)

                nmr = st.tile([128, 4], F32, tag="nmr")
                nc.vector.tensor_tensor(nmr[:], mv[:, :, 0], r[:], op=ALU.mult)
                nc.vector.tensor_scalar_mul(nmr[:], nmr[:], -1.0)
                xnb = wk.tile([128, 4, 256], BF16, tag="xnb")
                for j in range(4):
                    nc.scalar.activation(xnb[:, j, :], xg[:, j, :], AF.Identity,
                                         bias=nmr[:, j:j + 1], scale=r[:, j:j + 1])
                # transpose to channel-major
                in_cat = 16 <= r0 < 84
                if in_cat:
                    dst = xt
                    roff = r0 - 16
                else:
                    dst = wk1.tile([128, 2, 4, XT_W], BF16, tag="xnt")
                    roff = 0
                for j in range(4):
                    tp = ps.tile([128, 256], BF16, tag="mm")
                    for ch in range(2):
                        nc.tensor.transpose(tp[:, 128 * ch:128 * ch + 128],
                                            xnb[:, j, 128 * ch:128 * ch + 128], Ident[:])
                    nc.vector.tensor_copy(
                        dst[:, 0:2, roff + j, 1:129],
                        tp[:].rearrange("p (c b) -> p c b", c=2))
                # dwn conv on these 4 rows
                pd = ps.tile([128, 512], F32, tag="mm")
                pdv = pd[:].rearrange("p (a b) -> p a b", a=4)
                for ch in range(2):
                    nc.tensor.matmul(pdv, Wdwn[:, ch, :],
                                     dst[:, ch, roff:roff + 4, 1:129],
                                     start=(ch == 0), stop=(ch == 1))
                nc.scalar.activation(dx[:, r0:r0 + 4, 16:144], pdv, AF.Relu, bias=Bdwn)
                # im2col copies: flat dst offset applies the kx shift; row wrap
                # lands in the zero pads and the matmul read window skips it
                for (t, i, rs0, s) in IM2COL_SCHED.get(g, []):
                    eng = nc.sync if (t * ND + i) % 2 == 0 else nc.scalar
                    eng.dma_start(
                        sh_dram[t, 8 * i:8 * i + 8, 32 - s:32 - s + 72 * 160],
                        dx[8 * i:8 * i + 8, rs0:rs0 + 72, :])

            # (im2col writes were interleaved into the phase-1/2 loop above)

            # ============ phase 4: k/q/v convs + exp(kq) ============
            PARTIALS = []
            psums_t = st.tile([128, 20], F32, tag="psums", name="psums_t")
            for c9 in range(9):
                j0 = 8 * c9
                grps = [g for g in range(2) if j0 + 4 * g < KR]
                pk, pq, pv = {}, {}, {}
                for g in grps:
                    pk[g] = acc.tile([128, 512], F32, tag="acc", name=f"pk{c9}_{g}")
                    pq[g] = acc.tile([128, 512], F32, tag="acc", name=f"pq{c9}_{g}")
                    pv[g] = acc.tile([128, 512], F32, tag="acc", name=f"pv{c9}_{g}")
                for tg in range(3):
                    sht = shp.tile([128, 3, 8, 160], BF16, tag="sht")
                    eng = nc.sync if tg % 2 == 0 else nc.scalar
                    eng.dma_start(
                        sht[:].rearrange("p t a b -> p t (a b)"),
                        sh_dram[3 * tg:3 * tg + 3, :,
                                32 + j0 * 160:32 + (j0 + 8) * 160].rearrange(
                                    "t p f -> p t f"))
                    for tt in range(3):
                        t = 3 * tg + tt
                        for grp in grps:
                            rhs = sht[:, tt, 4 * grp:4 * grp + 4, 16:144]
                            nc.tensor.matmul(pk[grp][:], Wk[:, t, :], rhs,
                                             start=(t == 0), stop=(t == 8))
                            nc.tensor.matmul(pq[grp][:], Wq[:, t, :], rhs,
                                             start=(t == 0), stop=(t == 8))
                            nc.tensor.matmul(pv[grp][:], Wv[:, t, :], rhs,
                                             start=(t == 0), stop=(t == 8))
                for grp in grps:
                    jb = j0 + 4 * grp
                    nrows = min(4, KR - jb)
                    nn = 128 * nrows
                    kev = wk.tile([128, 4, 128], BF16, tag="kev")
                    qev = wk.tile([128, 4, 128], BF16, tag="qev")
                    nc.scalar.activation(kev[:, 0:nrows, :], pk[grp][:, 0:nn], AF.Relu,
                                         bias=Bk)
                    nc.scalar.activation(qev[:, 0:nrows, :], pq[grp][:, 0:nn], AF.Relu,
                                         bias=Bq)
                    nc.scalar.activation(vbuf[:, jb:jb + nrows, :], pv[grp][:, 0:nn],
                                         AF.Relu, bias=Bv)
                    kqf = wk.tile([128, 4, 128], F32, tag="kqf")
                    nc.vector.tensor_tensor(kqf[:, 0:nrows, :], kev[:, 0:nrows, :],
                                            qev[:, 0:nrows, :], op=ALU.mult)
                    nc.scalar.activation(ekq[:, jb:jb + nrows, :], kqf[:, 0:nrows, :],
                                         AF.Exp)
                    # partial softmax sum over the fresh rows restricted to [1, 65)
                    lo = max(jb, 1)
                    hi = min(jb + nrows, 65)
                    if lo < hi:
                        pidx = len(PARTIALS)
                        psl = psums_t[:, pidx:pidx + 1]
                        nc.vector.tensor_reduce(psl, ekq[:, lo:hi, :],
                                                axis=mybir.AxisListType.XY, op=ALU.add)
                        PARTIALS.append(pidx)

            # ============ phase 5: softmax sums + pairwise AllReduce ============
            sums = st.tile([128, 1], F32, tag="sums")
            nc.vector.tensor_reduce(sums[:], psums_t[:, 0:len(PARTIALS)],
                                    axis=mybir.AxisListType.X, op=ALU.add)
            nc.sync.dma_start(cc_in[:], sums[:])
            nc.gpsimd.collective_compute(
                "AllReduce", ALU.add,
                replica_groups=[[0, 1], [2, 3], [4, 5], [6, 7]],
                ins=[cc_in.opt()], outs=[cc_out.opt()],
            )
            tsum = st.tile([128, 1], F32, tag="tsum")
            nc.sync.dma_start(tsum[:], cc_out[:])
            rs = st.tile([128, 1], F32, tag="rs")
            nc.vector.reciprocal(rs[:], tsum[:])

            # ============ phase 6: attn*v + LN_up -> ybuf ============
            for c22 in range(22):
                j0 = 3 * c22
                avf = wk.tile([128, 3, 128], F32, tag="avf")
                nc.vector.tensor_tensor(avf[:], ekq[:, j0:j0 + 3, :],
                                        vbuf[:, j0:j0 + 3, :], op=ALU.mult)
                avb = wk.tile([128, 3, 128], BF16, tag="avb")
                nc.scalar.activation(avb[:], avf[:], AF.Identity, scale=rs[:])
                sqb = wk.tile([128, 3, 128], BF16, tag="sqb")
                nc.scalar.activation(sqb[:], avb[:], AF.Square)
                pm = acc.tile([128, 384], F32, tag="acc")
                pq2 = acc.tile([128, 384], F32, tag="acc")
                nc.tensor.matmul(pm[:], Gm[:], avb[:].rearrange("p a b -> p (a b)"),
                                 start=True, stop=True)
                nc.tensor.matmul(pq2[:], Gm[:], sqb[:].rearrange("p a b -> p (a b)"),
                                 start=True, stop=True)
                msb = wk1.tile([128, 384], F32, tag="msb")
                nc.scalar.copy(msb[:], pm[:])
                m2 = wk1.tile([128, 384], F32, tag="m2")
                nc.scalar.activation(m2[:], msb[:], AF.Square)
                varu = wk1.tile([128, 384], F32, tag="varu")
                nc.vector.tensor_tensor(varu[:], pq2[:], m2[:], op=ALU.subtract)
                nc.vector.tensor_scalar_add(varu[:], varu[:], EPS)
                ru = wk1.tile([128, 384], F32, tag="ru")
                nc.scalar.activation(ru[:], varu[:], AF.Abs_reciprocal_sqrt)
                yt = wk1.tile([128, 384], F32, tag="yt")
                nc.vector.tensor_tensor(yt[:], avb[:].rearrange("p a b -> p (a b)"),
                                        msb[:], op=ALU.subtract)
                nc.vector.tensor_tensor(
                    ybuf[:, j0:j0 + 3, 1:129],
                    yt[:].rearrange("p (a b) -> p a b", a=3),
                    ru[:].rearrange("p (a b) -> p a b", a=3), op=ALU.mult)

            # ============ phase 7+8: smooth conv + LN_out ============
            for pc in range(16):
                r0 = 4 * pc
                op_t = wk.tile([128, 2, 4, 128], BF16, tag="opt")
                for mc in range(2):
                    psm = ps.tile([128, 512], F32, tag="mm")
                    first = True
                    for t in range(9):
                        ky, kx = t // 3, t % 3
                        for kc in range(3):
                            if kc < 2:
                                rhs = xt[:, kc, r0 + ky:r0 + ky + 4, kx:kx + 128]
                            else:
                                rhs = ybuf[:, r0 + ky:r0 + ky + 4, kx:kx + 128]
                            nc.tensor.matmul(
                                psm[:], Ws[:, t, kc, 128 * mc:128 * mc + 128], rhs,
                                start=first, stop=(t == 8 and kc == 2))
                            first = False
                    nc.scalar.activation(op_t[:, mc, :, :],
                                         psm[:].rearrange("p (a b) -> p a b", a=4),
                                         AF.Relu, bias=Bias[:, 4 + mc:5 + mc])
                # transpose to pixel-major
                on_t = wk.tile([128, 4, 256], BF16, tag="ont")
                for j in range(4):
                    for mc in range(2):
                        tp = ps.tile([128, 256], BF16, tag="mm")
                        nc.tensor.transpose(tp[:, 0:128], op_t[:, mc, j, :], Ident[:])
                        nc.vector.tensor_copy(on_t[:, j, 128 * mc:128 * mc + 128],
                                              tp[:, 0:128])
                # LN_out (over 256 channels, free dim now)
                bsto = st.tile([128, 4, 6], F32, tag="bsto")
                for j in range(4):
                    nc.vector.bn_stats(bsto[:, j, :], on_t[:, j, :])
                mvo = st.tile([128, 4, 2], F32, tag="mvo")
                for j in range(4):
                    nc.vector.bn_aggr(mvo[:, j, :], bsto[:, j, :])
                sdo = st.tile([128, 4], F32, tag="sdo")
                nc.vector.tensor_scalar_add(sdo[:], mvo[:, :, 1], EPS)
                ro = st.tile([128, 4], F32, tag="ro")
                nc.scalar.activation(ro[:], sdo[:], AF.Abs_reciprocal_sqrt)
                orow = wk.tile([128, 4, 256], F32, tag="orow")
                for j in range(4):
                    nc.vector.tensor_scalar(orow[:, j, :], on_t[:, j, :],
                                            mvo[:, j, 0:1], ro[:, j:j + 1],
                                            op0=ALU.subtract, op1=ALU.mult)
                nc.gpsimd.dma_start(out.ap()[r0:r0 + 4].rearrange("r w c -> w r c"),
                                     orow[:])

    nc.compile()
    return nc


# --------------------------------------------------------------------------
# public entry point
# --------------------------------------------------------------------------
def kernel(**inputs):
    if "nc" not in _CACHE:
        _CACHE["nc"] = build_kernel()
    nc = _CACHE["nc"]

    wts = _prep_weights(inputs)
    shards = _shard_x(np.asarray(inputs["x"], np.float32))
    in_maps = []
    for c in range(N_CORES):
        m = {"x": shards[c]}
        m.update(wts)
        in_maps.append(m)
    res = run_bass_kernel_spmd(nc, in_maps, core_ids=list(range(N_CORES)))
    full = np.empty((N, H, W, C), np.float32)
    for c in range(N_CORES):
        n, v = c // 2, c % 2
        full[n, 64 * v:64 * v + 64] = res.results[c]["out"]
    return full


# revision 18
# speedup vs baseline: 2.2725x; 1.0320x over previous
"""Trainium2 Bass kernel for nn_AtrousAttentionBlock (16 dilated attention branches + smooth conv).

Sharding: 8 cores = (image n in [0,4)) x (vertical half v in [0,2)); core 2n+v computes
output rows [64v, 64v+64) of image n from a zero-padded 100-row x shard (17-row halos).
On-chip layout is channel-major bf16 (f32 PSUM accumulation):
  phase 1: LN_in (pixel-major stats) + PE transpose -> xn channel-major
  phase 2: 1x1 "dwn" conv, all 16 branches packed as 128 partitions (16br x 8ch)
  phase 3: per-branch dilation shifts materialized via DRAM im2col round trip
  phase 4: k/q/v 3x3 dilated convs as block-diagonal-weight matmuls (9 taps accumulated)
  phase 5: spatial-softmax partial sums + pairwise AllReduce (image halves)
  phase 6: attn*v + LN_up (stats via block-ones matmul, broadcast fused)
  phase 7: 3x3 smooth conv 384->256 (27 accumulating K-chunk matmuls per pixel tile)
  phase 8: LN_out in pixel-major after PE transpose, DMA out in NHWC
"""
import numpy as np
import ml_dtypes

import concourse.bass as bass
import concourse.bacc as bacc
import concourse.tile as tile
import concourse.mybir as mybir
from concourse.bass_utils import run_bass_kernel_spmd

N, H, W, C = 4, 128, 128, 256
ND, DF = 16, 8
CAT = C + ND * DF
EPS = 1e-3
N_CORES = 8

HALO = 17        # x halo rows per side (16 for dilation-16 taps + 1 for smooth conv)
SR = 100         # stored shard rows (98 used, padded to 4-row groups)
XT_W = 130       # padded width of channel-major xn/y (1 zero col per side)
XT_R = 68        # xn rows kept for the smooth conv (shard rows 16..84)
DX_R = 104       # dx rows ([0,98) real + zero tail so all 9x8-row kqv chunks stay in range)
DX_W = 160       # dx padded width (16 zero cols per side)
KR = 66          # kqv rows used: image rows 64v-1 .. 64v+65

F32 = mybir.dt.float32
BF16 = mybir.dt.bfloat16
AF = mybir.ActivationFunctionType
ALU = mybir.AluOpType

_CACHE = {}


# --------------------------------------------------------------------------
# host-side preparation
# --------------------------------------------------------------------------
def _prep_weights(inp):
    """Fold LN affines into conv weights; build PE-layout weight matrices."""
    for k in ("ln_in_b", "ln_up_b", "ln_out_b"):
        assert np.all(np.asarray(inp[k]) == 0), f"{k} != 0 unsupported"
    assert np.all(np.asarray(inp["ln_out_g"]) == 1), "ln_out_g != 1 unsupported"
    g_in = np.asarray(inp["ln_in_g"], np.float64)
    g_up = np.asarray(inp["ln_up_g"], np.float64)

    W_dwn = np.zeros((C, 128), np.float64)
    for i in range(ND):
        W_dwn[:, 8 * i:8 * i + 8] = inp["dwn_w"][i, 0, 0] * g_in[:, None]

    def blockdiag(w):  # [ND,3,3,DF,DF] -> [9,128,128]
        out = np.zeros((9, 128, 128), np.float64)
        for t in range(9):
            ky, kx = t // 3, t % 3
            for i in range(ND):
                out[t, 8 * i:8 * i + 8, 8 * i:8 * i + 8] = w[i, ky, kx]
        return out

    Ws = np.asarray(inp["smooth_w"], np.float64).copy()
    Ws[:, :, :C, :] *= g_in[None, None, :, None]
    for i in range(ND):
        Ws[:, :, C + 8 * i:C + 8 * i + 8, :] *= g_up[None, None, :, None]

    Gm = np.zeros((128, 128), np.float64)   # per-branch mean matrix (ones8x8 / 8)
    for i in range(ND):
        Gm[8 * i:8 * i + 8, 8 * i:8 * i + 8] = 1.0 / 8.0

    bf = lambda a: np.ascontiguousarray(np.asarray(a, np.float32)).astype(ml_dtypes.bfloat16)
    f32c = lambda a: np.ascontiguousarray(np.asarray(a, np.float32))
    biases = np.stack([
        np.asarray(inp["dwn_b"]).reshape(128),
        np.asarray(inp["k_b"]).reshape(128),
        np.asarray(inp["q_b"]).reshape(128),
        np.asarray(inp["v_b"]).reshape(128),
        np.asarray(inp["smooth_b"])[0:128],
        np.asarray(inp["smooth_b"])[128:256],
    ], axis=1)
    return dict(
        wdwn=bf(W_dwn),
        wk=bf(blockdiag(inp["k_w"])), wq=bf(blockdiag(inp["q_w"])), wv=bf(blockdiag(inp["v_w"])),
        ws=bf(Ws.reshape(9, 3, 128, 256)),
        biases=f32c(biases),
        gm=bf(Gm),
        ident=bf(np.eye(128)),
    )


def _shard_x(x):
    shards = []
    for c in range(N_CORES):
        n, v = c // 2, c % 2
        sh = np.zeros((SR, W, C), np.float32)
        lo, hi = 64 * v - HALO, 64 * v + 81
        slo, shi = max(0, lo), min(H, hi)
        sh[slo - lo:shi - lo] = x[n, slo:shi]
        shards.append(np.ascontiguousarray(sh))
    return shards


# --------------------------------------------------------------------------
# device kernel
# --------------------------------------------------------------------------
def build_kernel():
    nc = bacc.Bacc("TRN2", target_bir_lowering=False, debug=False, num_devices=N_CORES)

    x_in = nc.dram_tensor("x", [SR, W, C], F32, kind="ExternalInput")
    wdwn_in = nc.dram_tensor("wdwn", [C, 128], BF16, kind="ExternalInput")
    wk_in = nc.dram_tensor("wk", [9, 128, 128], BF16, kind="ExternalInput")
    wq_in = nc.dram_tensor("wq", [9, 128, 128], BF16, kind="ExternalInput")
    wv_in = nc.dram_tensor("wv", [9, 128, 128], BF16, kind="ExternalInput")
    ws_in = nc.dram_tensor("ws", [9, 3, 128, 256], BF16, kind="ExternalInput")
    bias_in = nc.dram_tensor("biases", [128, 6], F32, kind="ExternalInput")
    gm_in = nc.dram_tensor("gm", [128, 128], BF16, kind="ExternalInput")
    id_in = nc.dram_tensor("ident", [128, 128], BF16, kind="ExternalInput")
    out = nc.dram_tensor("out", [64, W, C], F32, kind="ExternalOutput")

    with tile.TileContext(nc) as tc:
        with (
            tc.tile_pool(name="const", bufs=1) as cp,
            tc.tile_pool(name="big", bufs=1) as bp,
            tc.tile_pool(name="work", bufs=2) as wk,
            tc.tile_pool(name="wk1", bufs=1) as wk1,
            tc.tile_pool(name="sht", bufs=2) as shp,
            tc.tile_pool(name="st", bufs=3) as st,
            tc.tile_pool(name="ps", bufs=2, space="PSUM") as ps,
            tc.tile_pool(name="acc", bufs=6, space="PSUM") as acc,
            tc.tile_pool(name="dram", bufs=1, space="DRAM") as dram,
        ):
            # ---- constants ----
            Wdwn = cp.tile([128, 2, 128], BF16, tag="wdwn")
            nc.sync.dma_start(Wdwn[:], wdwn_in.ap().rearrange("(c p) m -> p c m", p=128))
            Wk = cp.tile([128, 9, 128], BF16, tag="wk")
            Wq = cp.tile([128, 9, 128], BF16, tag="wq")
            Wv = cp.tile([128, 9, 128], BF16, tag="wv")
            Ws = cp.tile([128, 9, 3, 256], BF16, tag="ws")
            nc.sync.dma_start(Wk[:], wk_in.ap().rearrange("t p m -> p t m"))
            nc.sync.dma_start(Wq[:], wq_in.ap().rearrange("t p m -> p t m"))
            nc.sync.dma_start(Wv[:], wv_in.ap().rearrange("t p m -> p t m"))
            nc.sync.dma_start(Ws[:], ws_in.ap().rearrange("t k p m -> p t k m"))
            Bias = cp.tile([128, 6], F32, tag="bias")
            nc.sync.dma_start(Bias[:], bias_in.ap())
            Bdwn = Bias[:, 0:1]
            Bk = Bias[:, 1:2]
            Bq = Bias[:, 2:3]
            Bv = Bias[:, 3:4]
            Gm = cp.tile([128, 128], BF16, tag="gm")
            Ident = cp.tile([128, 128], BF16, tag="id")
            nc.sync.dma_start(Gm[:], gm_in.ap())
            nc.sync.dma_start(Ident[:], id_in.ap())

            # ---- persistent activations ----
            xt = bp.tile([128, 2, XT_R, XT_W], BF16, tag="xt")    # xn shard rows 16..84
            dx = bp.tile([128, DX_R, DX_W], BF16, tag="dx")
            ekq = bp.tile([128, KR, 128], BF16, tag="ekq")
            vbuf = bp.tile([128, KR, 128], BF16, tag="vbuf")
            ybuf = bp.tile([128, KR, XT_W], BF16, tag="ybuf")

            nc.vector.memset(xt[:, :, :, 0:1], 0.0)
            nc.vector.memset(xt[:, :, :, XT_W - 1:XT_W], 0.0)
            nc.vector.memset(dx[:, :, 0:16], 0.0)
            nc.vector.memset(dx[:, :, DX_W - 16:DX_W], 0.0)
            nc.vector.memset(dx[:, 98:DX_R, 16:DX_W - 16], 0.0)
            nc.vector.memset(ybuf[:, :, 0:1], 0.0)
            nc.vector.memset(ybuf[:, :, XT_W - 1:XT_W], 0.0)

            # per-(tap,branch) flat-offset copies: dst[32-s+f] = dx_flat[rs0*160+f]
            # read window [16:144) of each 160-pitch row then sees dx[rs0+j, c+s].
            sh_dram = dram.tile([9, 128, 32 + 72 * 160 + 32], BF16, tag="shd")
            cc_in = dram.tile([128, 1], F32, tag="ccin")
            cc_out = dram.tile([128, 1], F32, tag="ccout")

            # ============ phase 1+2: LN_in + transpose + dwn conv ============
            IM2COL_SCHED = {}
            for t in range(9):
                ky, kx = t // 3, t % 3
                for i in range(ND):
                    d = i + 1
                    rs0 = 16 + (ky - 1) * d
                    s = (kx - 1) * d
                    for half, (ra, rb) in enumerate(((0, 36), (36, 72))):
                        g_ready = max(0, (min(rs0 + rb, 98) + 3) // 4 - 1)
                        IM2COL_SCHED.setdefault(g_ready, []).append(
                            (t, i, rs0 + ra, s, ra))
            for g in range(SR // 4):
                r0 = 4 * g
                xg = wk.tile([128, 4, 256], F32, tag="xg")
                nc.gpsimd.dma_start(xg[:], x_in.ap()[r0:r0 + 4].rearrange("r w c -> w r c"))
                bst = st.tile([128, 4, 6], F32, tag="bst")
                for j in range(4):
                    nc.vector.bn_stats(bst[:, j, :], xg[:, j, :])
                mv = st.tile([128, 4, 2], F32, tag="mv")
                for j in range(4):
                    nc.vector.bn_aggr(mv[:, j, :], bst[:, j, :])
                sd = st.tile([128, 4], F32, tag="sd")
                nc.vector.tensor_scalar_add(sd[:], mv[:, :, 1], EPS)
                r = st.tile([128, 4], F32, tag="r")
                nc.scalar.activation(r[:], sd[:], AF.Abs_reciprocal_sqrt)

                nmr = st.tile([128, 4], F32, tag="nmr")
                nc.vector.tensor_tensor(nmr[:], mv[:, :, 0], r[:], op=ALU.mult)
                nc.vector.tensor_scalar_mul(nmr[:], nmr[:], -1.0)
                xnb = wk.tile([128, 4, 256], BF16, tag="xnb")
                for j in range(4):
                    nc.scalar.activation(xnb[:, j, :], xg[:, j, :], AF.Identity,
                                         bias=nmr[:, j:j + 1], scale=r[:, j:j + 1])
                # transpose to channel-major
                in_cat = 16 <= r0 < 84
                if in_cat:
                    dst = xt
                    roff = r0 - 16
                else:
                    dst = wk1.tile([128, 2, 4, XT_W], BF16, tag="xnt")
                    roff = 0
                for j in range(4):
                    tp = acc.tile([128, 256], BF16, tag="acc")
                    for ch in range(2):
                        nc.tensor.transpose(tp[:, 128 * ch:128 * ch + 128],
                                            xnb[:, j, 128 * ch:128 * ch + 128], Ident[:])
                    nc.vector.tensor_copy(
                        dst[:, 0:2, roff + j, 1:129],
                        tp[:].rearrange("p (c b) -> p c b", c=2))
                # dwn conv on these 4 rows
                pd = ps.tile([128, 512], F32, tag="mm")
                pdv = pd[:].rearrange("p (a b) -> p a b", a=4)
                for ch in range(2):
                    nc.tensor.matmul(pdv, Wdwn[:, ch, :],
                                     dst[:, ch, roff:roff + 4, 1:129],
                                     start=(ch == 0), stop=(ch == 1))
                nc.scalar.activation(dx[:, r0:r0 + 4, 16:144], pdv, AF.Relu, bias=Bdwn)
                # im2col copies: flat dst offset applies the kx shift; row wrap
                # lands in the zero pads and the matmul read window skips it
                for (t, i, r0w, s, ra) in IM2COL_SCHED.get(g, []):
                    eng = nc.sync if (t * ND + i) % 2 == 0 else nc.scalar
                    o0 = 32 - s + ra * 160
                    eng.dma_start(
                        sh_dram[t, 8 * i:8 * i + 8, o0:o0 + 36 * 160],
                        dx[8 * i:8 * i + 8, r0w:r0w + 36, :])

            # (im2col writes were interleaved into the phase-1/2 loop above)

            # ============ phase 4: k/q/v convs + exp(kq) ============
            PARTIALS = []
            psums_t = st.tile([128, 20], F32, tag="psums", name="psums_t")
            for c9 in range(9):
                j0 = 8 * c9
                grps = [g for g in range(2) if j0 + 4 * g < KR]
                pk, pq, pv = {}, {}, {}
                for g in grps:
                    pk[g] = acc.tile([128, 512], F32, tag="acc", name=f"pk{c9}_{g}")
                    pq[g] = acc.tile([128, 512], F32, tag="acc", name=f"pq{c9}_{g}")
                    pv[g] = acc.tile([128, 512], F32, tag="acc", name=f"pv{c9}_{g}")
                for tg in range(3):
                    sht = shp.tile([128, 3, 8, 160], BF16, tag="sht")
                    eng = nc.sync if tg % 2 == 0 else nc.scalar
                    eng.dma_start(
                        sht[:].rearrange("p t a b -> p t (a b)"),
                        sh_dram[3 * tg:3 * tg + 3, :,
                                32 + j0 * 160:32 + (j0 + 8) * 160].rearrange(
                                    "t p f -> p t f"))
                    for tt in range(3):
                        t = 3 * tg + tt
                        for grp in grps:
                            rhs = sht[:, tt, 4 * grp:4 * grp + 4, 16:144]
                            nc.tensor.matmul(pk[grp][:], Wk[:, t, :], rhs,
                                             start=(t == 0), stop=(t == 8))
                            nc.tensor.matmul(pq[grp][:], Wq[:, t, :], rhs,
                                             start=(t == 0), stop=(t == 8))
                            nc.tensor.matmul(pv[grp][:], Wv[:, t, :], rhs,
                                             start=(t == 0), stop=(t == 8))
                for grp in grps:
                    jb = j0 + 4 * grp
                    nrows = min(4, KR - jb)
                    nn = 128 * nrows
                    kev = wk.tile([128, 4, 128], BF16, tag="kev")
                    qev = wk.tile([128, 4, 128], BF16, tag="qev")
                    nc.scalar.activation(kev[:, 0:nrows, :], pk[grp][:, 0:nn], AF.Relu,
                                         bias=Bk)
                    nc.scalar.activation(qev[:, 0:nrows, :], pq[grp][:, 0:nn], AF.Relu,
                                         bias=Bq)
                    nc.scalar.activation(vbuf[:, jb:jb + nrows, :], pv[grp][:, 0:nn],
                                         AF.Relu, bias=Bv)
                    kqf = wk.tile([128, 4, 128], F32, tag="kqf")
                    nc.vector.tensor_tensor(kqf[:, 0:nrows, :], kev[:, 0:nrows, :],
                                            qev[:, 0:nrows, :], op=ALU.mult)
                    nc.scalar.activation(ekq[:, jb:jb + nrows, :], kqf[:, 0:nrows, :],
                                         AF.Exp)
                    # partial softmax sum over the fresh rows restricted to [1, 65)
                    lo = max(jb, 1)
                    hi = min(jb + nrows, 65)
                    if lo < hi:
                        pidx = len(PARTIALS)
                        psl = psums_t[:, pidx:pidx + 1]
                        nc.vector.tensor_reduce(psl, ekq[:, lo:hi, :],
                                                axis=mybir.AxisListType.XY, op=ALU.add)
                        PARTIALS.append(pidx)

            # ============ phase 5: softmax sums + pairwise AllReduce ============
            sums = st.tile([128, 1], F32, tag="sums")
            nc.vector.tensor_reduce(sums[:], psums_t[:, 0:len(PARTIALS)],
                                    axis=mybir.AxisListType.X, op=ALU.add)
            nc.sync.dma_start(cc_in[:], sums[:])
            nc.gpsimd.collective_compute(
                "AllReduce", ALU.add,
                replica_groups=[[0, 1], [2, 3], [4, 5], [6, 7]],
                ins=[cc_in.opt()], outs=[cc_out.opt()],
            )
            tsum = st.tile([128, 1], F32, tag="tsum")
            nc.sync.dma_start(tsum[:], cc_out[:])
            rs = st.tile([128, 1], F32, tag="rs")
            nc.vector.reciprocal(rs[:], tsum[:])

            # ============ phase 6: attn*v + LN_up -> ybuf ============
            for c22 in range(22):
                j0 = 3 * c22
                avf = wk.tile([128, 3, 128], F32, tag="avf")
                nc.vector.tensor_tensor(avf[:], ekq[:, j0:j0 + 3, :],
                                        vbuf[:, j0:j0 + 3, :], op=ALU.mult)
                avb = wk.tile([128, 3, 128], BF16, tag="avb")
                nc.scalar.activation(avb[:], avf[:], AF.Identity, scale=rs[:])
                sqb = wk.tile([128, 3, 128], BF16, tag="sqb")
                nc.scalar.activation(sqb[:], avb[:], AF.Square)
                pm = acc.tile([128, 384], F32, tag="acc")
                pq2 = acc.tile([128, 384], F32, tag="acc")
                nc.tensor.matmul(pm[:], Gm[:], avb[:].rearrange("p a b -> p (a b)"),
                                 start=True, stop=True)
                nc.tensor.matmul(pq2[:], Gm[:], sqb[:].rearrange("p a b -> p (a b)"),
                                 start=True, stop=True)
                msb = wk1.tile([128, 384], F32, tag="msb")
                nc.scalar.copy(msb[:], pm[:])
                m2 = wk1.tile([128, 384], F32, tag="m2")
                nc.scalar.activation(m2[:], msb[:], AF.Square)
                varu = wk1.tile([128, 384], F32, tag="varu")
                nc.vector.tensor_tensor(varu[:], pq2[:], m2[:], op=ALU.subtract)
                nc.vector.tensor_scalar_add(varu[:], varu[:], EPS)
                ru = wk1.tile([128, 384], F32, tag="ru")
                nc.scalar.activation(ru[:], varu[:], AF.Abs_reciprocal_sqrt)
                yt = wk1.tile([128, 384], F32, tag="yt")
                nc.vector.tensor_tensor(yt[:], avb[:].rearrange("p a b -> p (a b)"),
                                        msb[:], op=ALU.subtract)
                nc.vector.tensor_tensor(
                    ybuf[:, j0:j0 + 3, 1:129],
                    yt[:].rearrange("p (a b) -> p a b", a=3),
                    ru[:].rearrange("p (a b) -> p a b", a=3), op=ALU.mult)

            # ============ phase 7+8: smooth conv + LN_out ============
            for pc in range(16):
                r0 = 4 * pc
                op_t = wk.tile([128, 2, 4, 128], BF16, tag="opt")
                for mc in range(2):
                    psm = ps.tile([128, 512], F32, tag="mm")
                    first = True
                    for t in range(9):
                        ky, kx = t // 3, t % 3
                        for kc in range(3):
                            if kc < 2:
                                rhs = xt[:, kc, r0 + ky:r0 + ky + 4, kx:kx + 128]
                            else:
                                rhs = ybuf[:, r0 + ky:r0 + ky + 4, kx:kx + 128]
                            nc.tensor.matmul(
                                psm[:], Ws[:, t, kc, 128 * mc:128 * mc + 128], rhs,
                                start=first, stop=(t == 8 and kc == 2))
                            first = False
                    nc.scalar.activation(op_t[:, mc, :, :],
                                         psm[:].rearrange("p (a b) -> p a b", a=4),
                                         AF.Relu, bias=Bias[:, 4 + mc:5 + mc])
                # transpose to pixel-major
                on_t = wk.tile([128, 4, 256], BF16, tag="ont")
                for j in range(4):
                    for mc in range(2):
                        tp = acc.tile([128, 256], BF16, tag="acc")
                        nc.tensor.transpose(tp[:, 0:128], op_t[:, mc, j, :], Ident[:])
                        nc.vector.tensor_copy(on_t[:, j, 128 * mc:128 * mc + 128],
                                              tp[:, 0:128])
                # LN_out (over 256 channels, free dim now)
                bsto = st.tile([128, 4, 6], F32, tag="bsto")
                for j in range(4):
                    nc.vector.bn_stats(bsto[:, j, :], on_t[:, j, :])
                mvo = st.tile([128, 4, 2], F32, tag="mvo")
                for j in range(4):
                    nc.vector.bn_aggr(mvo[:, j, :], bsto[:, j, :])
                sdo = st.tile([128, 4], F32, tag="sdo")
                nc.vector.tensor_scalar_add(sdo[:], mvo[:, :, 1], EPS)
                ro = st.tile([128, 4], F32, tag="ro")
                nc.scalar.activation(ro[:], sdo[:], AF.Abs_reciprocal_sqrt)
                orow = wk.tile([128, 4, 256], F32, tag="orow")
                for j in range(4):
                    nc.vector.tensor_scalar(orow[:, j, :], on_t[:, j, :],
                                            mvo[:, j, 0:1], ro[:, j:j + 1],
                                            op0=ALU.subtract, op1=ALU.mult)
                nc.gpsimd.dma_start(out.ap()[r0:r0 + 4].rearrange("r w c -> w r c"),
                                     orow[:])

    nc.compile()
    return nc


# --------------------------------------------------------------------------
# public entry point
# --------------------------------------------------------------------------
def kernel(**inputs):
    if "nc" not in _CACHE:
        _CACHE["nc"] = build_kernel()
    nc = _CACHE["nc"]

    wts = _prep_weights(inputs)
    shards = _shard_x(np.asarray(inputs["x"], np.float32))
    in_maps = []
    for c in range(N_CORES):
        m = {"x": shards[c]}
        m.update(wts)
        in_maps.append(m)
    res = run_bass_kernel_spmd(nc, in_maps, core_ids=list(range(N_CORES)))
    full = np.empty((N, H, W, C), np.float32)
    for c in range(N_CORES):
        n, v = c // 2, c % 2
        full[n, 64 * v:64 * v + 64] = res.results[c]["out"]
    return full


# revision 20
# speedup vs baseline: 2.2845x; 1.0053x over previous
"""Trainium2 Bass kernel for nn_AtrousAttentionBlock (16 dilated attention branches + smooth conv).

Sharding: 8 cores = (image n in [0,4)) x (vertical half v in [0,2)); core 2n+v computes
output rows [64v, 64v+64) of image n from a zero-padded 100-row x shard (17-row halos).
On-chip layout is channel-major bf16 (f32 PSUM accumulation):
  phase 1: LN_in (pixel-major stats) + PE transpose -> xn channel-major
  phase 2: 1x1 "dwn" conv, all 16 branches packed as 128 partitions (16br x 8ch)
  phase 3: per-branch dilation shifts materialized via DRAM im2col round trip
  phase 4: k/q/v 3x3 dilated convs as block-diagonal-weight matmuls (9 taps accumulated)
  phase 5: spatial-softmax partial sums + pairwise AllReduce (image halves)
  phase 6: attn*v + LN_up (stats via block-ones matmul, broadcast fused)
  phase 7: 3x3 smooth conv 384->256 (27 accumulating K-chunk matmuls per pixel tile)
  phase 8: LN_out in pixel-major after PE transpose, DMA out in NHWC
"""
import numpy as np
import ml_dtypes

import concourse.bass as bass
import concourse.bacc as bacc
import concourse.tile as tile
import concourse.mybir as mybir
from concourse.bass_utils import run_bass_kernel_spmd

N, H, W, C = 4, 128, 128, 256
ND, DF = 16, 8
CAT = C + ND * DF
EPS = 1e-3
N_CORES = 8

HALO = 17        # x halo rows per side (16 for dilation-16 taps + 1 for smooth conv)
SR = 104         # stored shard rows (98 used, padded to 8-row groups)
XT_W = 130       # padded width of channel-major xn/y (1 zero col per side)
XT_R = 72        # xn rows kept for the smooth conv (shard rows 16..88, 8-row aligned)
DX_R = 104       # dx rows ([0,98) real + zero tail so all 9x8-row kqv chunks stay in range)
DX_W = 160       # dx padded width (16 zero cols per side)
KR = 66          # kqv rows used: image rows 64v-1 .. 64v+65

F32 = mybir.dt.float32
BF16 = mybir.dt.bfloat16
AF = mybir.ActivationFunctionType
ALU = mybir.AluOpType

_CACHE = {}


# --------------------------------------------------------------------------
# host-side preparation
# --------------------------------------------------------------------------
def _prep_weights(inp):
    """Fold LN affines into conv weights; build PE-layout weight matrices."""
    for k in ("ln_in_b", "ln_up_b", "ln_out_b"):
        assert np.all(np.asarray(inp[k]) == 0), f"{k} != 0 unsupported"
    assert np.all(np.asarray(inp["ln_out_g"]) == 1), "ln_out_g != 1 unsupported"
    g_in = np.asarray(inp["ln_in_g"], np.float64)
    g_up = np.asarray(inp["ln_up_g"], np.float64)

    W_dwn = np.zeros((C, 128), np.float64)
    for i in range(ND):
        W_dwn[:, 8 * i:8 * i + 8] = inp["dwn_w"][i, 0, 0] * g_in[:, None]

    def blockdiag(w):  # [ND,3,3,DF,DF] -> [9,128,128]
        out = np.zeros((9, 128, 128), np.float64)
        for t in range(9):
            ky, kx = t // 3, t % 3
            for i in range(ND):
                out[t, 8 * i:8 * i + 8, 8 * i:8 * i + 8] = w[i, ky, kx]
        return out

    Ws = np.asarray(inp["smooth_w"], np.float64).copy()
    Ws[:, :, :C, :] *= g_in[None, None, :, None]
    for i in range(ND):
        Ws[:, :, C + 8 * i:C + 8 * i + 8, :] *= g_up[None, None, :, None]

    Gm = np.zeros((128, 128), np.float64)   # per-branch mean matrix (ones8x8 / 8)
    for i in range(ND):
        Gm[8 * i:8 * i + 8, 8 * i:8 * i + 8] = 1.0 / 8.0

    bf = lambda a: np.ascontiguousarray(np.asarray(a, np.float32)).astype(ml_dtypes.bfloat16)
    f32c = lambda a: np.ascontiguousarray(np.asarray(a, np.float32))
    biases = np.stack([
        np.asarray(inp["dwn_b"]).reshape(128),
        np.asarray(inp["k_b"]).reshape(128),
        np.asarray(inp["q_b"]).reshape(128),
        np.asarray(inp["v_b"]).reshape(128),
        np.asarray(inp["smooth_b"])[0:128],
        np.asarray(inp["smooth_b"])[128:256],
    ], axis=1)
    return dict(
        wdwn=bf(W_dwn),
        wk=bf(blockdiag(inp["k_w"])), wq=bf(blockdiag(inp["q_w"])), wv=bf(blockdiag(inp["v_w"])),
        ws=bf(Ws.reshape(9, 3, 128, 256)),
        biases=f32c(biases),
        gm=bf(Gm),
        ident=bf(np.eye(128)),
    )


def _shard_x(x):
    shards = []
    for c in range(N_CORES):
        n, v = c // 2, c % 2
        sh = np.zeros((SR, W, C), np.float32)
        lo, hi = 64 * v - HALO, 64 * v + 81
        slo, shi = max(0, lo), min(H, hi)
        sh[slo - lo:shi - lo] = x[n, slo:shi]
        shards.append(np.ascontiguousarray(sh))
    return shards


# --------------------------------------------------------------------------
# device kernel
# --------------------------------------------------------------------------
def build_kernel():
    nc = bacc.Bacc("TRN2", target_bir_lowering=False, debug=False, num_devices=N_CORES)

    x_in = nc.dram_tensor("x", [SR, W, C], F32, kind="ExternalInput")
    wdwn_in = nc.dram_tensor("wdwn", [C, 128], BF16, kind="ExternalInput")
    wk_in = nc.dram_tensor("wk", [9, 128, 128], BF16, kind="ExternalInput")
    wq_in = nc.dram_tensor("wq", [9, 128, 128], BF16, kind="ExternalInput")
    wv_in = nc.dram_tensor("wv", [9, 128, 128], BF16, kind="ExternalInput")
    ws_in = nc.dram_tensor("ws", [9, 3, 128, 256], BF16, kind="ExternalInput")
    bias_in = nc.dram_tensor("biases", [128, 6], F32, kind="ExternalInput")
    gm_in = nc.dram_tensor("gm", [128, 128], BF16, kind="ExternalInput")
    id_in = nc.dram_tensor("ident", [128, 128], BF16, kind="ExternalInput")
    out = nc.dram_tensor("out", [64, W, C], F32, kind="ExternalOutput")

    with tile.TileContext(nc) as tc:
        with (
            tc.tile_pool(name="const", bufs=1) as cp,
            tc.tile_pool(name="big", bufs=1) as bp,
            tc.tile_pool(name="work", bufs=2) as wk,
            tc.tile_pool(name="wk1", bufs=1) as wk1,
            tc.tile_pool(name="sht", bufs=2) as shp,
            tc.tile_pool(name="st", bufs=3) as st,
            tc.tile_pool(name="ps", bufs=2, space="PSUM") as ps,
            tc.tile_pool(name="acc", bufs=6, space="PSUM") as acc,
            tc.tile_pool(name="dram", bufs=1, space="DRAM") as dram,
        ):
            # ---- constants ----
            Wdwn = cp.tile([128, 2, 128], BF16, tag="wdwn")
            nc.sync.dma_start(Wdwn[:], wdwn_in.ap().rearrange("(c p) m -> p c m", p=128))
            Wk = cp.tile([128, 9, 128], BF16, tag="wk")
            Wq = cp.tile([128, 9, 128], BF16, tag="wq")
            Wv = cp.tile([128, 9, 128], BF16, tag="wv")
            Ws = cp.tile([128, 9, 3, 256], BF16, tag="ws")
            nc.sync.dma_start(Wk[:], wk_in.ap().rearrange("t p m -> p t m"))
            nc.sync.dma_start(Wq[:], wq_in.ap().rearrange("t p m -> p t m"))
            nc.sync.dma_start(Wv[:], wv_in.ap().rearrange("t p m -> p t m"))
            nc.sync.dma_start(Ws[:], ws_in.ap().rearrange("t k p m -> p t k m"))
            Bias = cp.tile([128, 6], F32, tag="bias")
            nc.sync.dma_start(Bias[:], bias_in.ap())
            Bdwn = Bias[:, 0:1]
            Bk = Bias[:, 1:2]
            Bq = Bias[:, 2:3]
            Bv = Bias[:, 3:4]
            Gm = cp.tile([128, 128], BF16, tag="gm")
            Ident = cp.tile([128, 128], BF16, tag="id")
            nc.sync.dma_start(Gm[:], gm_in.ap())
            nc.sync.dma_start(Ident[:], id_in.ap())

            # ---- persistent activations ----
            xt = bp.tile([128, 2, XT_R, XT_W], BF16, tag="xt")    # xn shard rows 16..84
            ekq = bp.tile([128, KR, 128], BF16, tag="ekq")
            vbuf = bp.tile([128, KR, 128], BF16, tag="vbuf")
            ybuf = bp.tile([128, KR, XT_W], BF16, tag="ybuf")

            nc.vector.memset(xt[:, :, :, 0:1], 0.0)
            nc.vector.memset(xt[:, :, :, XT_W - 1:XT_W], 0.0)
            nc.vector.memset(ybuf[:, :, 0:1], 0.0)
            nc.vector.memset(ybuf[:, :, XT_W - 1:XT_W], 0.0)

            # dx lives in DRAM; per-branch tap shifts are linear in branch index so
            # the whole im2col is 9 arithmetic-stride DRAM->DRAM copies.
            dx_dram = dram.tile([128, DX_R, DX_W], BF16, tag="dxd")
            sh_dram = dram.tile([9, 128, 32 + 72 * 160 + 32], BF16, tag="shd")
            # zero-fill the 16-col side pads of dx_dram once
            zpad = wk1.tile([128, 104, 16], BF16, tag="zpad")
            nc.vector.memset(zpad[:], 0.0)
            nc.sync.dma_start(dx_dram[:, :, 0:16], zpad[:])
            nc.sync.dma_start(dx_dram[:, :, DX_W - 16:DX_W], zpad[:])
            cc_in = dram.tile([128, 1], F32, tag="ccin")
            cc_out = dram.tile([128, 1], F32, tag="ccout")

            # ============ phase 1+2: LN_in + transpose + dwn conv ============
            for g in range(SR // 8):
                r0 = 8 * g
                xg = wk.tile([128, 8, 256], F32, tag="xg")
                nc.gpsimd.dma_start(xg[:], x_in.ap()[r0:r0 + 8].rearrange("r w c -> w r c"))
                bst = st.tile([128, 8, 6], F32, tag="bst")
                for j in range(8):
                    nc.vector.bn_stats(bst[:, j, :], xg[:, j, :])
                mv = st.tile([128, 8, 2], F32, tag="mv")
                for j in range(8):
                    nc.vector.bn_aggr(mv[:, j, :], bst[:, j, :])
                sd = st.tile([128, 8], F32, tag="sd")
                nc.vector.tensor_scalar_add(sd[:], mv[:, :, 1], EPS)
                r = st.tile([128, 8], F32, tag="r")
                nc.scalar.activation(r[:], sd[:], AF.Abs_reciprocal_sqrt)

                nmr = st.tile([128, 8], F32, tag="nmr")
                nc.vector.tensor_tensor(nmr[:], mv[:, :, 0], r[:], op=ALU.mult)
                nc.vector.tensor_scalar_mul(nmr[:], nmr[:], -1.0)
                xnb = wk.tile([128, 8, 256], BF16, tag="xnb")
                for j in range(8):
                    nc.scalar.activation(xnb[:, j, :], xg[:, j, :], AF.Identity,
                                         bias=nmr[:, j:j + 1], scale=r[:, j:j + 1])
                # transpose to channel-major (xt covers shard rows 16..88, 8-aligned)
                in_cat = 16 <= r0 < 88
                if in_cat:
                    dst = xt
                    roff = r0 - 16
                else:
                    dst = wk1.tile([128, 2, 8, XT_W], BF16, tag="xnt")
                    roff = 0
                for j in range(8):
                    tp = acc.tile([128, 256], BF16, tag="acc")
                    for ch in range(2):
                        nc.tensor.transpose(tp[:, 128 * ch:128 * ch + 128],
                                            xnb[:, j, 128 * ch:128 * ch + 128], Ident[:])
                    nc.vector.tensor_copy(
                        dst[:, 0:2, roff + j, 1:129],
                        tp[:].rearrange("p (c b) -> p c b", c=2))
                # dwn conv on these 8 rows (two 4-row psums)
                dxs = wk.tile([128, 8, 160], BF16, tag="dxs")
                for half in range(2):
                    pd = ps.tile([128, 512], F32, tag="mm")
                    pdv = pd[:].rearrange("p (a b) -> p a b", a=4)
                    for ch in range(2):
                        nc.tensor.matmul(pdv, Wdwn[:, ch, :],
                                         dst[:, ch, roff + 4 * half:roff + 4 * half + 4, 1:129],
                                         start=(ch == 0), stop=(ch == 1))
                    nc.scalar.activation(dxs[:, 4 * half:4 * half + 4, 16:144], pdv,
                                         AF.Relu, bias=Bdwn)
                nc.vector.memset(dxs[:, :, 0:16], 0.0)
                nc.vector.memset(dxs[:, :, 144:160], 0.0)
                nc.sync.dma_start(dx_dram[:, r0:r0 + 8, :], dxs[:])

            # ============ phase 3: im2col as 9 arithmetic-stride DRAM copies ====
            PD = DX_R * DX_W          # dx_dram partition stride (elements)
            PS = 11584                # sh_dram per-partition plane size
            for t in range(9):
                ky, kx = t // 3, t % 3
                srcf = bass.AP(dx_dram[:].tensor, (16 + (ky - 1)) * DX_W,
                               [[8 * PD + (ky - 1) * DX_W, 16], [PD, 8],
                                [1, 72 * 160]])
                dstf = bass.AP(sh_dram[:].tensor,
                               t * 128 * PS + 32 - (kx - 1),
                               [[8 * PS - (kx - 1), 16], [PS, 8],
                                [1, 72 * 160]])
                eng = nc.sync if t % 2 == 0 else nc.scalar
                eng.dma_start(dstf, srcf)

            # (im2col writes were interleaved into the phase-1/2 loop above)

            # ============ phase 4: k/q/v convs + exp(kq) ============
            PARTIALS = []
            psums_t = st.tile([128, 20], F32, tag="psums", name="psums_t")
            for c9 in range(9):
                j0 = 8 * c9
                grps = [g for g in range(2) if j0 + 4 * g < KR]
                pk, pq, pv = {}, {}, {}
                for g in grps:
                    pk[g] = acc.tile([128, 512], F32, tag="acc", name=f"pk{c9}_{g}")
                    pq[g] = acc.tile([128, 512], F32, tag="acc", name=f"pq{c9}_{g}")
                    pv[g] = acc.tile([128, 512], F32, tag="acc", name=f"pv{c9}_{g}")
                for tg in range(3):
                    sht = shp.tile([128, 3, 8, 160], BF16, tag="sht")
                    eng = nc.sync if tg % 2 == 0 else nc.scalar
                    eng.dma_start(
                        sht[:].rearrange("p t a b -> p t (a b)"),
                        sh_dram[3 * tg:3 * tg + 3, :,
                                32 + j0 * 160:32 + (j0 + 8) * 160].rearrange(
                                    "t p f -> p t f"))
                    for tt in range(3):
                        t = 3 * tg + tt
                        for grp in grps:
                            rhs = sht[:, tt, 4 * grp:4 * grp + 4, 16:144]
                            nc.tensor.matmul(pk[grp][:], Wk[:, t, :], rhs,
                                             start=(t == 0), stop=(t == 8))
                            nc.tensor.matmul(pq[grp][:], Wq[:, t, :], rhs,
                                             start=(t == 0), stop=(t == 8))
                            nc.tensor.matmul(pv[grp][:], Wv[:, t, :], rhs,
                                             start=(t == 0), stop=(t == 8))
                for grp in grps:
                    jb = j0 + 4 * grp
                    nrows = min(4, KR - jb)
                    nn = 128 * nrows
                    kev = wk.tile([128, 4, 128], BF16, tag="kev")
                    qev = wk.tile([128, 4, 128], BF16, tag="qev")
                    nc.scalar.activation(kev[:, 0:nrows, :], pk[grp][:, 0:nn], AF.Relu,
                                         bias=Bk)
                    nc.scalar.activation(qev[:, 0:nrows, :], pq[grp][:, 0:nn], AF.Relu,
                                         bias=Bq)
                    nc.scalar.activation(vbuf[:, jb:jb + nrows, :], pv[grp][:, 0:nn],
                                         AF.Relu, bias=Bv)
                    kqf = wk.tile([128, 4, 128], F32, tag="kqf")
                    nc.vector.tensor_tensor(kqf[:, 0:nrows, :], kev[:, 0:nrows, :],
                                            qev[:, 0:nrows, :], op=ALU.mult)
                    nc.scalar.activation(ekq[:, jb:jb + nrows, :], kqf[:, 0:nrows, :],
                                         AF.Exp)
                    # partial softmax sum over the fresh rows restricted to [1, 65)
                    lo = max(jb, 1)
                    hi = min(jb + nrows, 65)
                    if lo < hi:
                        pidx = len(PARTIALS)
                        psl = psums_t[:, pidx:pidx + 1]
                        nc.vector.tensor_reduce(psl, ekq[:, lo:hi, :],
                                                axis=mybir.AxisListType.XY, op=ALU.add)
                        PARTIALS.append(pidx)

            # ============ phase 5: softmax sums + pairwise AllReduce ============
            sums = st.tile([128, 1], F32, tag="sums")
            nc.vector.tensor_reduce(sums[:], psums_t[:, 0:len(PARTIALS)],
                                    axis=mybir.AxisListType.X, op=ALU.add)
            nc.sync.dma_start(cc_in[:], sums[:])
            nc.gpsimd.collective_compute(
                "AllReduce", ALU.add,
                replica_groups=[[0, 1], [2, 3], [4, 5], [6, 7]],
                ins=[cc_in.opt()], outs=[cc_out.opt()],
            )
            tsum = st.tile([128, 1], F32, tag="tsum")
            nc.sync.dma_start(tsum[:], cc_out[:])
            rs = st.tile([128, 1], F32, tag="rs")
            nc.vector.reciprocal(rs[:], tsum[:])

            # ============ phase 6: attn*v + LN_up -> ybuf ============
            for c22 in range(22):
                j0 = 3 * c22
                avf = wk.tile([128, 3, 128], F32, tag="avf")
                nc.vector.tensor_tensor(avf[:], ekq[:, j0:j0 + 3, :],
                                        vbuf[:, j0:j0 + 3, :], op=ALU.mult)
                avb = wk.tile([128, 3, 128], BF16, tag="avb")
                nc.scalar.activation(avb[:], avf[:], AF.Identity, scale=rs[:])
                sqb = wk.tile([128, 3, 128], BF16, tag="sqb")
                nc.scalar.activation(sqb[:], avb[:], AF.Square)
                pm = acc.tile([128, 384], F32, tag="acc")
                pq2 = acc.tile([128, 384], F32, tag="acc")
                nc.tensor.matmul(pm[:], Gm[:], avb[:].rearrange("p a b -> p (a b)"),
                                 start=True, stop=True)
                nc.tensor.matmul(pq2[:], Gm[:], sqb[:].rearrange("p a b -> p (a b)"),
                                 start=True, stop=True)
                msb = wk1.tile([128, 384], F32, tag="msb")
                nc.scalar.copy(msb[:], pm[:])
                m2 = wk1.tile([128, 384], F32, tag="m2")
                nc.scalar.activation(m2[:], msb[:], AF.Square)
                varu = wk1.tile([128, 384], F32, tag="varu")
                nc.vector.tensor_tensor(varu[:], pq2[:], m2[:], op=ALU.subtract)
                nc.vector.tensor_scalar_add(varu[:], varu[:], EPS)
                ru = wk1.tile([128, 384], F32, tag="ru")
                nc.scalar.activation(ru[:], varu[:], AF.Abs_reciprocal_sqrt)
                yt = wk1.tile([128, 384], F32, tag="yt")
                nc.vector.tensor_tensor(yt[:], avb[:].rearrange("p a b -> p (a b)"),
                                        msb[:], op=ALU.subtract)
                nc.vector.tensor_tensor(
                    ybuf[:, j0:j0 + 3, 1:129],
                    yt[:].rearrange("p (a b) -> p a b", a=3),
                    ru[:].rearrange("p (a b) -> p a b", a=3), op=ALU.mult)

            # ============ phase 7+8: smooth conv + LN_out ============
            for pc in range(16):
                r0 = 4 * pc
                op_t = wk.tile([128, 2, 4, 128], BF16, tag="opt")
                for mc in range(2):
                    psm = ps.tile([128, 512], F32, tag="mm")
                    first = True
                    for t in range(9):
                        ky, kx = t // 3, t % 3
                        for kc in range(3):
                            if kc < 2:
                                rhs = xt[:, kc, r0 + ky:r0 + ky + 4, kx:kx + 128]
                            else:
                                rhs = ybuf[:, r0 + ky:r0 + ky + 4, kx:kx + 128]
                            nc.tensor.matmul(
                                psm[:], Ws[:, t, kc, 128 * mc:128 * mc + 128], rhs,
                                start=first, stop=(t == 8 and kc == 2))
                            first = False
                    nc.scalar.activation(op_t[:, mc, :, :],
                                         psm[:].rearrange("p (a b) -> p a b", a=4),
                                         AF.Relu, bias=Bias[:, 4 + mc:5 + mc])
                # transpose to pixel-major
                on_t = wk.tile([128, 4, 256], BF16, tag="ont")
                for j in range(4):
                    for mc in range(2):
                        tp = acc.tile([128, 256], BF16, tag="acc")
                        nc.tensor.transpose(tp[:, 0:128], op_t[:, mc, j, :], Ident[:])
                        nc.vector.tensor_copy(on_t[:, j, 128 * mc:128 * mc + 128],
                                              tp[:, 0:128])
                # LN_out (over 256 channels, free dim now)
                bsto = st.tile([128, 4, 6], F32, tag="bsto")
                for j in range(4):
                    nc.vector.bn_stats(bsto[:, j, :], on_t[:, j, :])
                mvo = st.tile([128, 4, 2], F32, tag="mvo")
                for j in range(4):
                    nc.vector.bn_aggr(mvo[:, j, :], bsto[:, j, :])
                sdo = st.tile([128, 4], F32, tag="sdo")
                nc.vector.tensor_scalar_add(sdo[:], mvo[:, :, 1], EPS)
                ro = st.tile([128, 4], F32, tag="ro")
                nc.scalar.activation(ro[:], sdo[:], AF.Abs_reciprocal_sqrt)
                orow = wk.tile([128, 4, 256], F32, tag="orow")
                for j in range(4):
                    nc.vector.tensor_scalar(orow[:, j, :], on_t[:, j, :],
                                            mvo[:, j, 0:1], ro[:, j:j + 1],
                                            op0=ALU.subtract, op1=ALU.mult)
                nc.gpsimd.dma_start(out.ap()[r0:r0 + 4].rearrange("r w c -> w r c"),
                                     orow[:])

    nc.compile()
    return nc


# --------------------------------------------------------------------------
# public entry point
# --------------------------------------------------------------------------
def kernel(**inputs):
    if "nc" not in _CACHE:
        _CACHE["nc"] = build_kernel()
    nc = _CACHE["nc"]

    wts = _prep_weights(inputs)
    shards = _shard_x(np.asarray(inputs["x"], np.float32))
    in_maps = []
    for c in range(N_CORES):
        m = {"x": shards[c]}
        m.update(wts)
        in_maps.append(m)
    res = run_bass_kernel_spmd(nc, in_maps, core_ids=list(range(N_CORES)))
    full = np.empty((N, H, W, C), np.float32)
    for c in range(N_CORES):
        n, v = c // 2, c % 2
        full[n, 64 * v:64 * v + 64] = res.results[c]["out"]
    return full


# revision 21
# speedup vs baseline: 2.3408x; 1.0247x over previous
"""Trainium2 Bass kernel for nn_AtrousAttentionBlock (16 dilated attention branches + smooth conv).

Sharding: 8 cores = (image n in [0,4)) x (vertical half v in [0,2)); core 2n+v computes
output rows [64v, 64v+64) of image n from a zero-padded 100-row x shard (17-row halos).
On-chip layout is channel-major bf16 (f32 PSUM accumulation):
  phase 1: LN_in (pixel-major stats) + PE transpose -> xn channel-major
  phase 2: 1x1 "dwn" conv, all 16 branches packed as 128 partitions (16br x 8ch)
  phase 3: per-branch dilation shifts materialized via DRAM im2col round trip
  phase 4: k/q/v 3x3 dilated convs as block-diagonal-weight matmuls (9 taps accumulated)
  phase 5: spatial-softmax partial sums + pairwise AllReduce (image halves)
  phase 6: attn*v + LN_up (stats via block-ones matmul, broadcast fused)
  phase 7: 3x3 smooth conv 384->256 (27 accumulating K-chunk matmuls per pixel tile)
  phase 8: LN_out in pixel-major after PE transpose, DMA out in NHWC
"""
import numpy as np
import ml_dtypes

import concourse.bass as bass
import concourse.bacc as bacc
import concourse.tile as tile
import concourse.mybir as mybir
from concourse.bass_utils import run_bass_kernel_spmd

N, H, W, C = 4, 128, 128, 256
ND, DF = 16, 8
CAT = C + ND * DF
EPS = 1e-3
N_CORES = 8

HALO = 17        # x halo rows per side (16 for dilation-16 taps + 1 for smooth conv)
SR = 104         # stored shard rows (98 used, padded to 8-row groups)
XT_W = 130       # padded width of channel-major xn/y (1 zero col per side)
XT_R = 72        # xn rows kept for the smooth conv (shard rows 16..88, 8-row aligned)
DX_R = 104       # dx rows ([0,98) real + zero tail so all 9x8-row kqv chunks stay in range)
DX_W = 160       # dx padded width (16 zero cols per side)
KR = 66          # kqv rows used: image rows 64v-1 .. 64v+65

F32 = mybir.dt.float32
BF16 = mybir.dt.bfloat16
AF = mybir.ActivationFunctionType
ALU = mybir.AluOpType

_CACHE = {}


# --------------------------------------------------------------------------
# host-side preparation
# --------------------------------------------------------------------------
def _prep_weights(inp):
    """Fold LN affines into conv weights; build PE-layout weight matrices."""
    for k in ("ln_in_b", "ln_up_b", "ln_out_b"):
        assert np.all(np.asarray(inp[k]) == 0), f"{k} != 0 unsupported"
    assert np.all(np.asarray(inp["ln_out_g"]) == 1), "ln_out_g != 1 unsupported"
    g_in = np.asarray(inp["ln_in_g"], np.float64)
    g_up = np.asarray(inp["ln_up_g"], np.float64)

    W_dwn = np.zeros((C, 128), np.float64)
    for i in range(ND):
        W_dwn[:, 8 * i:8 * i + 8] = inp["dwn_w"][i, 0, 0] * g_in[:, None]

    def blockdiag(w):  # [ND,3,3,DF,DF] -> [9,128,128]
        out = np.zeros((9, 128, 128), np.float64)
        for t in range(9):
            ky, kx = t // 3, t % 3
            for i in range(ND):
                out[t, 8 * i:8 * i + 8, 8 * i:8 * i + 8] = w[i, ky, kx]
        return out

    Ws = np.asarray(inp["smooth_w"], np.float64).copy()
    Ws[:, :, :C, :] *= g_in[None, None, :, None]
    for i in range(ND):
        Ws[:, :, C + 8 * i:C + 8 * i + 8, :] *= g_up[None, None, :, None]

    Gm = np.zeros((128, 128), np.float64)   # per-branch mean matrix (ones8x8 / 8)
    for i in range(ND):
        Gm[8 * i:8 * i + 8, 8 * i:8 * i + 8] = 1.0 / 8.0

    bf = lambda a: np.ascontiguousarray(np.asarray(a, np.float32)).astype(ml_dtypes.bfloat16)
    f32c = lambda a: np.ascontiguousarray(np.asarray(a, np.float32))
    biases = np.stack([
        np.asarray(inp["dwn_b"]).reshape(128),
        np.asarray(inp["k_b"]).reshape(128),
        np.asarray(inp["q_b"]).reshape(128),
        np.asarray(inp["v_b"]).reshape(128),
        np.asarray(inp["smooth_b"])[0:128],
        np.asarray(inp["smooth_b"])[128:256],
    ], axis=1)
    return dict(
        wdwn=bf(W_dwn),
        wk=bf(blockdiag(inp["k_w"])), wq=bf(blockdiag(inp["q_w"])), wv=bf(blockdiag(inp["v_w"])),
        ws=bf(Ws.reshape(9, 3, 128, 256)),
        biases=f32c(biases),
        gm=bf(Gm),
        ident=bf(np.eye(128)),
    )


def _shard_x(x):
    shards = []
    for c in range(N_CORES):
        n, v = c // 2, c % 2
        sh = np.zeros((SR, W, C), np.float32)
        lo, hi = 64 * v - HALO, 64 * v + 81
        slo, shi = max(0, lo), min(H, hi)
        sh[slo - lo:shi - lo] = x[n, slo:shi]
        shards.append(np.ascontiguousarray(sh))
    return shards


# --------------------------------------------------------------------------
# device kernel
# --------------------------------------------------------------------------
def build_kernel():
    nc = bacc.Bacc("TRN2", target_bir_lowering=False, debug=False, num_devices=N_CORES)

    x_in = nc.dram_tensor("x", [SR, W, C], F32, kind="ExternalInput")
    wdwn_in = nc.dram_tensor("wdwn", [C, 128], BF16, kind="ExternalInput")
    wk_in = nc.dram_tensor("wk", [9, 128, 128], BF16, kind="ExternalInput")
    wq_in = nc.dram_tensor("wq", [9, 128, 128], BF16, kind="ExternalInput")
    wv_in = nc.dram_tensor("wv", [9, 128, 128], BF16, kind="ExternalInput")
    ws_in = nc.dram_tensor("ws", [9, 3, 128, 256], BF16, kind="ExternalInput")
    bias_in = nc.dram_tensor("biases", [128, 6], F32, kind="ExternalInput")
    gm_in = nc.dram_tensor("gm", [128, 128], BF16, kind="ExternalInput")
    id_in = nc.dram_tensor("ident", [128, 128], BF16, kind="ExternalInput")
    out = nc.dram_tensor("out", [64, W, C], F32, kind="ExternalOutput")

    with tile.TileContext(nc) as tc:
        with (
            tc.tile_pool(name="const", bufs=1) as cp,
            tc.tile_pool(name="big", bufs=1) as bp,
            tc.tile_pool(name="work", bufs=2) as wk,
            tc.tile_pool(name="wk1", bufs=1) as wk1,
            tc.tile_pool(name="sht", bufs=2) as shp,
            tc.tile_pool(name="st", bufs=3) as st,
            tc.tile_pool(name="ps", bufs=2, space="PSUM") as ps,
            tc.tile_pool(name="acc", bufs=6, space="PSUM") as acc,
            tc.tile_pool(name="dram", bufs=1, space="DRAM") as dram,
        ):
            # ---- constants ----
            Wdwn = cp.tile([128, 2, 128], BF16, tag="wdwn")
            nc.sync.dma_start(Wdwn[:], wdwn_in.ap().rearrange("(c p) m -> p c m", p=128))
            Wk = cp.tile([128, 9, 128], BF16, tag="wk")
            Wq = cp.tile([128, 9, 128], BF16, tag="wq")
            Wv = cp.tile([128, 9, 128], BF16, tag="wv")
            Ws = cp.tile([128, 9, 3, 256], BF16, tag="ws")
            nc.sync.dma_start(Wk[:], wk_in.ap().rearrange("t p m -> p t m"))
            nc.sync.dma_start(Wq[:], wq_in.ap().rearrange("t p m -> p t m"))
            nc.sync.dma_start(Wv[:], wv_in.ap().rearrange("t p m -> p t m"))
            nc.sync.dma_start(Ws[:], ws_in.ap().rearrange("t k p m -> p t k m"))
            Bias = cp.tile([128, 6], F32, tag="bias")
            nc.sync.dma_start(Bias[:], bias_in.ap())
            Bdwn = Bias[:, 0:1]
            Bk = Bias[:, 1:2]
            Bq = Bias[:, 2:3]
            Bv = Bias[:, 3:4]
            Gm = cp.tile([128, 128], BF16, tag="gm")
            Ident = cp.tile([128, 128], BF16, tag="id")
            nc.sync.dma_start(Gm[:], gm_in.ap())
            nc.sync.dma_start(Ident[:], id_in.ap())

            # ---- persistent activations ----
            xt = bp.tile([128, 2, XT_R, XT_W], BF16, tag="xt")    # xn shard rows 16..84
            ekq = bp.tile([128, KR, 128], BF16, tag="ekq")
            vbuf = bp.tile([128, KR, 128], BF16, tag="vbuf")
            ybuf = bp.tile([128, KR, XT_W], BF16, tag="ybuf")

            nc.vector.memset(xt[:, :, :, 0:1], 0.0)
            nc.vector.memset(xt[:, :, :, XT_W - 1:XT_W], 0.0)
            nc.vector.memset(ybuf[:, :, 0:1], 0.0)
            nc.vector.memset(ybuf[:, :, XT_W - 1:XT_W], 0.0)

            # dx lives in DRAM; per-branch tap shifts are linear in branch index so
            # the whole im2col is 9 arithmetic-stride DRAM->DRAM copies.
            dx_dram = dram.tile([128, DX_R, DX_W], BF16, tag="dxd")
            sh_dram = dram.tile([9, 128, 32 + 72 * 160 + 32], BF16, tag="shd")
            # zero-fill the 16-col side pads of dx_dram once
            zpad = wk1.tile([128, 104, 16], BF16, tag="zpad")
            nc.vector.memset(zpad[:], 0.0)
            nc.sync.dma_start(dx_dram[:, :, 0:16], zpad[:])
            nc.sync.dma_start(dx_dram[:, :, DX_W - 16:DX_W], zpad[:])
            cc_in = dram.tile([128, 1], F32, tag="ccin")
            cc_out = dram.tile([128, 1], F32, tag="ccout")

            # ============ phase 1+2: LN_in + transpose + dwn conv ============
            PD = DX_R * DX_W          # dx_dram partition stride (elements)
            SH_P = 11584              # sh_dram per-partition plane size
            IM2COL_SCHED = {}
            for t in range(9):
                ky = t // 3
                for T in range(3):
                    last_row = 16 + (ky - 1) * 16 + 24 * T + 24   # worst-case branch d=16
                    if ky == 0:
                        last_row = 24 * T + 24 + 16               # d=1..16: max rs0 = 15
                    g_ready = min((min(last_row, 104) + 7) // 8 - 1, SR // 8 - 1)
                    IM2COL_SCHED.setdefault(max(0, g_ready), []).append((t, T))
            for g in range(SR // 8):
                r0 = 8 * g
                xg = wk.tile([128, 8, 256], F32, tag="xg")
                nc.gpsimd.dma_start(xg[:], x_in.ap()[r0:r0 + 8].rearrange("r w c -> w r c"))
                bst = st.tile([128, 8, 6], F32, tag="bst")
                for j in range(8):
                    nc.vector.bn_stats(bst[:, j, :], xg[:, j, :])
                mv = st.tile([128, 8, 2], F32, tag="mv")
                for j in range(8):
                    nc.vector.bn_aggr(mv[:, j, :], bst[:, j, :])
                sd = st.tile([128, 8], F32, tag="sd")
                nc.vector.tensor_scalar_add(sd[:], mv[:, :, 1], EPS)
                r = st.tile([128, 8], F32, tag="r")
                nc.scalar.activation(r[:], sd[:], AF.Abs_reciprocal_sqrt)

                nmr = st.tile([128, 8], F32, tag="nmr")
                nc.vector.tensor_tensor(nmr[:], mv[:, :, 0], r[:], op=ALU.mult)
                nc.vector.tensor_scalar_mul(nmr[:], nmr[:], -1.0)
                xnb = wk.tile([128, 8, 256], BF16, tag="xnb")
                for j in range(8):
                    nc.scalar.activation(xnb[:, j, :], xg[:, j, :], AF.Identity,
                                         bias=nmr[:, j:j + 1], scale=r[:, j:j + 1])
                # transpose to channel-major (xt covers shard rows 16..88, 8-aligned)
                in_cat = 16 <= r0 < 88
                if in_cat:
                    dst = xt
                    roff = r0 - 16
                else:
                    dst = wk1.tile([128, 2, 8, XT_W], BF16, tag="xnt")
                    roff = 0
                for j in range(8):
                    tp = ps.tile([128, 256], BF16, tag="mm")
                    for ch in range(2):
                        nc.tensor.transpose(tp[:, 128 * ch:128 * ch + 128],
                                            xnb[:, j, 128 * ch:128 * ch + 128], Ident[:])
                    nc.vector.tensor_copy(
                        dst[:, 0:2, roff + j, 1:129],
                        tp[:].rearrange("p (c b) -> p c b", c=2))
                # dwn conv on these 8 rows (two 4-row psums)
                dxs = wk.tile([128, 8, 160], BF16, tag="dxs")
                for half in range(2):
                    pd = ps.tile([128, 512], F32, tag="mm")
                    pdv = pd[:].rearrange("p (a b) -> p a b", a=4)
                    for ch in range(2):
                        nc.tensor.matmul(pdv, Wdwn[:, ch, :],
                                         dst[:, ch, roff + 4 * half:roff + 4 * half + 4, 1:129],
                                         start=(ch == 0), stop=(ch == 1))
                    nc.scalar.activation(dxs[:, 4 * half:4 * half + 4, 16:144], pdv,
                                         AF.Relu, bias=Bdwn)
                nc.vector.memset(dxs[:, :, 0:16], 0.0)
                nc.vector.memset(dxs[:, :, 144:160], 0.0)
                nc.sync.dma_start(dx_dram[:, r0:r0 + 8, :], dxs[:])
                # im2col row-third copies whose source rows just completed
                for (t, T) in IM2COL_SCHED.get(g, []):
                    ky, kx = t // 3, t % 3
                    srcf = bass.AP(dx_dram[:].tensor,
                                   (16 + (ky - 1) + 24 * T) * DX_W,
                                   [[8 * PD + (ky - 1) * DX_W, 16], [PD, 8],
                                    [1, 24 * 160]])
                    dstf = bass.AP(sh_dram[:].tensor,
                                   t * 128 * SH_P + 32 - (kx - 1) + 24 * T * 160,
                                   [[8 * SH_P - (kx - 1), 16], [SH_P, 8],
                                    [1, 24 * 160]])
                    nc.sync.dma_start(dstf, srcf)

            # (im2col copies are interleaved into the phase-1/2 loop as row-thirds)

            # (im2col writes were interleaved into the phase-1/2 loop above)

            # ============ phase 4: k/q/v convs + exp(kq) ============
            PARTIALS = []
            psums_t = st.tile([128, 20], F32, tag="psums", name="psums_t")
            for c9 in range(9):
                j0 = 8 * c9
                grps = [g for g in range(2) if j0 + 4 * g < KR]
                pk, pq, pv = {}, {}, {}
                for g in grps:
                    pk[g] = acc.tile([128, 512], F32, tag="acc", name=f"pk{c9}_{g}")
                    pq[g] = acc.tile([128, 512], F32, tag="acc", name=f"pq{c9}_{g}")
                    pv[g] = acc.tile([128, 512], F32, tag="acc", name=f"pv{c9}_{g}")
                for tg in range(3):
                    sht = shp.tile([128, 3, 8, 160], BF16, tag="sht")
                    eng = nc.sync if tg % 2 == 0 else nc.scalar
                    eng.dma_start(
                        sht[:].rearrange("p t a b -> p t (a b)"),
                        sh_dram[3 * tg:3 * tg + 3, :,
                                32 + j0 * 160:32 + (j0 + 8) * 160].rearrange(
                                    "t p f -> p t f"))
                    for tt in range(3):
                        t = 3 * tg + tt
                        for grp in grps:
                            rhs = sht[:, tt, 4 * grp:4 * grp + 4, 16:144]
                            nc.tensor.matmul(pk[grp][:], Wk[:, t, :], rhs,
                                             start=(t == 0), stop=(t == 8))
                            nc.tensor.matmul(pq[grp][:], Wq[:, t, :], rhs,
                                             start=(t == 0), stop=(t == 8))
                            nc.tensor.matmul(pv[grp][:], Wv[:, t, :], rhs,
                                             start=(t == 0), stop=(t == 8))
                for grp in grps:
                    jb = j0 + 4 * grp
                    nrows = min(4, KR - jb)
                    nn = 128 * nrows
                    kev = wk.tile([128, 4, 128], BF16, tag="kev")
                    qev = wk.tile([128, 4, 128], BF16, tag="qev")
                    nc.scalar.activation(kev[:, 0:nrows, :], pk[grp][:, 0:nn], AF.Relu,
                                         bias=Bk)
                    nc.scalar.activation(qev[:, 0:nrows, :], pq[grp][:, 0:nn], AF.Relu,
                                         bias=Bq)
                    nc.scalar.activation(vbuf[:, jb:jb + nrows, :], pv[grp][:, 0:nn],
                                         AF.Relu, bias=Bv)
                    kqf = wk.tile([128, 4, 128], F32, tag="kqf")
                    nc.vector.tensor_tensor(kqf[:, 0:nrows, :], kev[:, 0:nrows, :],
                                            qev[:, 0:nrows, :], op=ALU.mult)
                    nc.scalar.activation(ekq[:, jb:jb + nrows, :], kqf[:, 0:nrows, :],
                                         AF.Exp)
                    # partial softmax sum over the fresh rows restricted to [1, 65)
                    lo = max(jb, 1)
                    hi = min(jb + nrows, 65)
                    if lo < hi:
                        pidx = len(PARTIALS)
                        psl = psums_t[:, pidx:pidx + 1]
                        nc.vector.tensor_reduce(psl, ekq[:, lo:hi, :],
                                                axis=mybir.AxisListType.XY, op=ALU.add)
                        PARTIALS.append(pidx)

            # ============ phase 5: softmax sums + pairwise AllReduce ============
            sums = st.tile([128, 1], F32, tag="sums")
            nc.vector.tensor_reduce(sums[:], psums_t[:, 0:len(PARTIALS)],
                                    axis=mybir.AxisListType.X, op=ALU.add)
            nc.sync.dma_start(cc_in[:], sums[:])
            nc.gpsimd.collective_compute(
                "AllReduce", ALU.add,
                replica_groups=[[0, 1], [2, 3], [4, 5], [6, 7]],
                ins=[cc_in.opt()], outs=[cc_out.opt()],
            )
            tsum = st.tile([128, 1], F32, tag="tsum")
            nc.sync.dma_start(tsum[:], cc_out[:])
            rs = st.tile([128, 1], F32, tag="rs")
            nc.vector.reciprocal(rs[:], tsum[:])

            # ============ phase 6: attn*v + LN_up -> ybuf ============
            for c22 in range(22):
                j0 = 3 * c22
                avf = wk.tile([128, 3, 128], F32, tag="avf")
                nc.vector.tensor_tensor(avf[:], ekq[:, j0:j0 + 3, :],
                                        vbuf[:, j0:j0 + 3, :], op=ALU.mult)
                avb = wk.tile([128, 3, 128], BF16, tag="avb")
                nc.scalar.activation(avb[:], avf[:], AF.Identity, scale=rs[:])
                sqb = wk.tile([128, 3, 128], BF16, tag="sqb")
                nc.scalar.activation(sqb[:], avb[:], AF.Square)
                pm = acc.tile([128, 384], F32, tag="acc")
                pq2 = acc.tile([128, 384], F32, tag="acc")
                nc.tensor.matmul(pm[:], Gm[:], avb[:].rearrange("p a b -> p (a b)"),
                                 start=True, stop=True)
                nc.tensor.matmul(pq2[:], Gm[:], sqb[:].rearrange("p a b -> p (a b)"),
                                 start=True, stop=True)
                msb = wk1.tile([128, 384], F32, tag="msb")
                nc.scalar.copy(msb[:], pm[:])
                m2 = wk1.tile([128, 384], F32, tag="m2")
                nc.scalar.activation(m2[:], msb[:], AF.Square)
                varu = wk1.tile([128, 384], F32, tag="varu")
                nc.vector.tensor_tensor(varu[:], pq2[:], m2[:], op=ALU.subtract)
                nc.vector.tensor_scalar_add(varu[:], varu[:], EPS)
                ru = wk1.tile([128, 384], F32, tag="ru")
                nc.scalar.activation(ru[:], varu[:], AF.Abs_reciprocal_sqrt)
                yt = wk1.tile([128, 384], F32, tag="yt")
                nc.vector.tensor_tensor(yt[:], avb[:].rearrange("p a b -> p (a b)"),
                                        msb[:], op=ALU.subtract)
                nc.vector.tensor_tensor(
                    ybuf[:, j0:j0 + 3, 1:129],
                    yt[:].rearrange("p (a b) -> p a b", a=3),
                    ru[:].rearrange("p (a b) -> p a b", a=3), op=ALU.mult)

            # ============ phase 7+8: smooth conv + LN_out ============
            for pc in range(16):
                r0 = 4 * pc
                op_t = wk.tile([128, 2, 4, 128], BF16, tag="opt")
                for mc in range(2):
                    psm = ps.tile([128, 512], F32, tag="mm")
                    first = True
                    for t in range(9):
                        ky, kx = t // 3, t % 3
                        for kc in range(3):
                            if kc < 2:
                                rhs = xt[:, kc, r0 + ky:r0 + ky + 4, kx:kx + 128]
                            else:
                                rhs = ybuf[:, r0 + ky:r0 + ky + 4, kx:kx + 128]
                            nc.tensor.matmul(
                                psm[:], Ws[:, t, kc, 128 * mc:128 * mc + 128], rhs,
                                start=first, stop=(t == 8 and kc == 2))
                            first = False
                    nc.scalar.activation(op_t[:, mc, :, :],
                                         psm[:].rearrange("p (a b) -> p a b", a=4),
                                         AF.Relu, bias=Bias[:, 4 + mc:5 + mc])
                # transpose to pixel-major
                on_t = wk.tile([128, 4, 256], BF16, tag="ont")
                for j in range(4):
                    for mc in range(2):
                        tp = acc.tile([128, 256], BF16, tag="acc")
                        nc.tensor.transpose(tp[:, 0:128], op_t[:, mc, j, :], Ident[:])
                        nc.vector.tensor_copy(on_t[:, j, 128 * mc:128 * mc + 128],
                                              tp[:, 0:128])
                # LN_out (over 256 channels, free dim now)
                bsto = st.tile([128, 4, 6], F32, tag="bsto")
                for j in range(4):
                    nc.vector.bn_stats(bsto[:, j, :], on_t[:, j, :])
                mvo = st.tile([128, 4, 2], F32, tag="mvo")
                for j in range(4):
                    nc.vector.bn_aggr(mvo[:, j, :], bsto[:, j, :])
                sdo = st.tile([128, 4], F32, tag="sdo")
                nc.vector.tensor_scalar_add(sdo[:], mvo[:, :, 1], EPS)
                ro = st.tile([128, 4], F32, tag="ro")
                nc.scalar.activation(ro[:], sdo[:], AF.Abs_reciprocal_sqrt)
                orow = wk.tile([128, 4, 256], F32, tag="orow")
                for j in range(4):
                    nc.vector.tensor_scalar(orow[:, j, :], on_t[:, j, :],
                                            mvo[:, j, 0:1], ro[:, j:j + 1],
                                            op0=ALU.subtract, op1=ALU.mult)
                nc.gpsimd.dma_start(out.ap()[r0:r0 + 4].rearrange("r w c -> w r c"),
                                     orow[:])

    nc.compile()
    return nc


# --------------------------------------------------------------------------
# public entry point
# --------------------------------------------------------------------------
def kernel(**inputs):
    if "nc" not in _CACHE:
        _CACHE["nc"] = build_kernel()
    nc = _CACHE["nc"]

    wts = _prep_weights(inputs)
    shards = _shard_x(np.asarray(inputs["x"], np.float32))
    in_maps = []
    for c in range(N_CORES):
        m = {"x": shards[c]}
        m.update(wts)
        in_maps.append(m)
    res = run_bass_kernel_spmd(nc, in_maps, core_ids=list(range(N_CORES)))
    full = np.empty((N, H, W, C), np.float32)
    for c in range(N_CORES):
        n, v = c // 2, c % 2
        full[n, 64 * v:64 * v + 64] = res.results[c]["out"]
    return full


# revision 23
# speedup vs baseline: 2.4143x; 1.0314x over previous
"""Trainium2 Bass kernel for nn_AtrousAttentionBlock (16 dilated attention branches + smooth conv).

Sharding: 8 cores = (image n in [0,4)) x (vertical half v in [0,2)); core 2n+v computes
output rows [64v, 64v+64) of image n from a zero-padded 100-row x shard (17-row halos).
On-chip layout is channel-major bf16 (f32 PSUM accumulation):
  phase 1: LN_in (pixel-major stats) + PE transpose -> xn channel-major
  phase 2: 1x1 "dwn" conv, all 16 branches packed as 128 partitions (16br x 8ch)
  phase 3: per-branch dilation shifts materialized via DRAM im2col round trip
  phase 4: k/q/v 3x3 dilated convs as block-diagonal-weight matmuls (9 taps accumulated)
  phase 5: spatial-softmax partial sums + pairwise AllReduce (image halves)
  phase 6: attn*v + LN_up (stats via block-ones matmul, broadcast fused)
  phase 7: 3x3 smooth conv 384->256 (27 accumulating K-chunk matmuls per pixel tile)
  phase 8: LN_out in pixel-major after PE transpose, DMA out in NHWC
"""
import numpy as np
import ml_dtypes

import concourse.bass as bass
import concourse.bacc as bacc
import concourse.tile as tile
import concourse.mybir as mybir
from concourse.bass_utils import run_bass_kernel_spmd

N, H, W, C = 4, 128, 128, 256
ND, DF = 16, 8
CAT = C + ND * DF
EPS = 1e-3
N_CORES = 8

HALO = 17        # x halo rows per side (16 for dilation-16 taps + 1 for smooth conv)
SR = 104         # stored shard rows (98 used, padded to 8-row groups)
XT_W = 130       # padded width of channel-major xn/y (1 zero col per side)
XT_R = 72        # xn rows kept for the smooth conv (shard rows 16..88, 8-row aligned)
DX_R = 104       # dx rows ([0,98) real + zero tail so all 9x8-row kqv chunks stay in range)
DX_W = 160       # dx padded width (16 zero cols per side)
KR = 66          # kqv rows used: image rows 64v-1 .. 64v+65

F32 = mybir.dt.float32
BF16 = mybir.dt.bfloat16
AF = mybir.ActivationFunctionType
ALU = mybir.AluOpType

_CACHE = {}


# --------------------------------------------------------------------------
# host-side preparation
# --------------------------------------------------------------------------
def _prep_weights(inp):
    """Fold LN affines into conv weights; build PE-layout weight matrices."""
    for k in ("ln_in_b", "ln_up_b", "ln_out_b"):
        assert np.all(np.asarray(inp[k]) == 0), f"{k} != 0 unsupported"
    assert np.all(np.asarray(inp["ln_out_g"]) == 1), "ln_out_g != 1 unsupported"
    g_in = np.asarray(inp["ln_in_g"], np.float64)
    g_up = np.asarray(inp["ln_up_g"], np.float64)

    W_dwn = np.zeros((C, 128), np.float64)
    for i in range(ND):
        W_dwn[:, 8 * i:8 * i + 8] = inp["dwn_w"][i, 0, 0] * g_in[:, None]

    def blockdiag(w):  # [ND,3,3,DF,DF] -> [9,128,128]
        out = np.zeros((9, 128, 128), np.float64)
        for t in range(9):
            ky, kx = t // 3, t % 3
            for i in range(ND):
                out[t, 8 * i:8 * i + 8, 8 * i:8 * i + 8] = w[i, ky, kx]
        return out

    Ws = np.asarray(inp["smooth_w"], np.float64).copy()
    Ws[:, :, :C, :] *= g_in[None, None, :, None]
    for i in range(ND):
        Ws[:, :, C + 8 * i:C + 8 * i + 8, :] *= g_up[None, None, :, None]

    Gm = np.zeros((128, 128), np.float64)   # per-branch mean matrix (ones8x8 / 8)
    for i in range(ND):
        Gm[8 * i:8 * i + 8, 8 * i:8 * i + 8] = 1.0 / 8.0

    bf = lambda a: np.ascontiguousarray(np.asarray(a, np.float32)).astype(ml_dtypes.bfloat16)
    f32c = lambda a: np.ascontiguousarray(np.asarray(a, np.float32))
    biases = np.stack([
        np.asarray(inp["dwn_b"]).reshape(128),
        np.asarray(inp["k_b"]).reshape(128),
        np.asarray(inp["q_b"]).reshape(128),
        np.asarray(inp["v_b"]).reshape(128),
        np.asarray(inp["smooth_b"])[0:128],
        np.asarray(inp["smooth_b"])[128:256],
    ], axis=1)
    return dict(
        wdwn=bf(W_dwn),
        wk=bf(blockdiag(inp["k_w"])), wq=bf(blockdiag(inp["q_w"])), wv=bf(blockdiag(inp["v_w"])),
        ws=bf(Ws.reshape(9, 3, 128, 256)),
        biases=f32c(biases),
        gm=bf(Gm),
        ident=bf(np.eye(128)),
    )


def _shard_x(x):
    shards = []
    for c in range(N_CORES):
        n, v = c // 2, c % 2
        sh = np.zeros((SR, W, C), np.float32)
        lo, hi = 64 * v - HALO, 64 * v + 81
        slo, shi = max(0, lo), min(H, hi)
        sh[slo - lo:shi - lo] = x[n, slo:shi]
        shards.append(np.ascontiguousarray(sh))
    return shards


# --------------------------------------------------------------------------
# device kernel
# --------------------------------------------------------------------------
def build_kernel():
    nc = bacc.Bacc("TRN2", target_bir_lowering=False, debug=False, num_devices=N_CORES)

    x_in = nc.dram_tensor("x", [SR, W, C], F32, kind="ExternalInput")
    wdwn_in = nc.dram_tensor("wdwn", [C, 128], BF16, kind="ExternalInput")
    wk_in = nc.dram_tensor("wk", [9, 128, 128], BF16, kind="ExternalInput")
    wq_in = nc.dram_tensor("wq", [9, 128, 128], BF16, kind="ExternalInput")
    wv_in = nc.dram_tensor("wv", [9, 128, 128], BF16, kind="ExternalInput")
    ws_in = nc.dram_tensor("ws", [9, 3, 128, 256], BF16, kind="ExternalInput")
    bias_in = nc.dram_tensor("biases", [128, 6], F32, kind="ExternalInput")
    gm_in = nc.dram_tensor("gm", [128, 128], BF16, kind="ExternalInput")
    id_in = nc.dram_tensor("ident", [128, 128], BF16, kind="ExternalInput")
    out = nc.dram_tensor("out", [64, W, C], F32, kind="ExternalOutput")

    with tile.TileContext(nc) as tc:
        with (
            tc.tile_pool(name="const", bufs=1) as cp,
            tc.tile_pool(name="big", bufs=1) as bp,
            tc.tile_pool(name="work", bufs=2) as wk,
            tc.tile_pool(name="wk3", bufs=3) as wk3,
            tc.tile_pool(name="wk1", bufs=1) as wk1,
            tc.tile_pool(name="sht", bufs=2) as shp,
            tc.tile_pool(name="st", bufs=3) as st,
            tc.tile_pool(name="ps", bufs=2, space="PSUM") as ps,
            tc.tile_pool(name="acc", bufs=6, space="PSUM") as acc,
            tc.tile_pool(name="dram", bufs=1, space="DRAM") as dram,
        ):
            # ---- constants ----
            Wdwn = cp.tile([128, 2, 128], BF16, tag="wdwn")
            nc.sync.dma_start(Wdwn[:], wdwn_in.ap().rearrange("(c p) m -> p c m", p=128))
            Wk = cp.tile([128, 9, 128], BF16, tag="wk")
            Wq = cp.tile([128, 9, 128], BF16, tag="wq")
            Wv = cp.tile([128, 9, 128], BF16, tag="wv")
            Ws = cp.tile([128, 9, 3, 256], BF16, tag="ws")
            nc.sync.dma_start(Wk[:], wk_in.ap().rearrange("t p m -> p t m"))
            nc.sync.dma_start(Wq[:], wq_in.ap().rearrange("t p m -> p t m"))
            nc.sync.dma_start(Wv[:], wv_in.ap().rearrange("t p m -> p t m"))
            nc.sync.dma_start(Ws[:], ws_in.ap().rearrange("t k p m -> p t k m"))
            Bias = cp.tile([128, 6], F32, tag="bias")
            nc.sync.dma_start(Bias[:], bias_in.ap())
            Bdwn = Bias[:, 0:1]
            Bk = Bias[:, 1:2]
            Bq = Bias[:, 2:3]
            Bv = Bias[:, 3:4]
            Gm = cp.tile([128, 128], BF16, tag="gm")
            Ident = cp.tile([128, 128], BF16, tag="id")
            nc.sync.dma_start(Gm[:], gm_in.ap())
            nc.sync.dma_start(Ident[:], id_in.ap())

            # ---- persistent activations ----
            xt = bp.tile([128, 2, XT_R, XT_W], BF16, tag="xt")    # xn shard rows 16..84
            ekq = bp.tile([128, KR, 128], BF16, tag="ekq")
            vbuf = bp.tile([128, KR, 128], BF16, tag="vbuf")
            ybuf = bp.tile([128, KR, XT_W], BF16, tag="ybuf")

            nc.vector.memset(xt[:, :, :, 0:1], 0.0)
            nc.vector.memset(xt[:, :, :, XT_W - 1:XT_W], 0.0)
            nc.vector.memset(ybuf[:, :, 0:1], 0.0)
            nc.vector.memset(ybuf[:, :, XT_W - 1:XT_W], 0.0)

            # dx lives in DRAM; per-branch tap shifts are linear in branch index so
            # the whole im2col is 9 arithmetic-stride DRAM->DRAM copies.
            dx_dram = dram.tile([128, DX_R, DX_W], BF16, tag="dxd")
            sh_dram = dram.tile([9, 128, 32 + 72 * 160 + 32], BF16, tag="shd")
            # zero-fill the 16-col side pads of dx_dram once
            zpad = wk1.tile([128, 104, 16], BF16, tag="zpad")
            nc.vector.memset(zpad[:], 0.0)
            nc.sync.dma_start(dx_dram[:, :, 0:16], zpad[:])
            nc.sync.dma_start(dx_dram[:, :, DX_W - 16:DX_W], zpad[:])
            cc_in = dram.tile([128, 1], F32, tag="ccin")
            cc_out = dram.tile([128, 1], F32, tag="ccout")

            # ============ phase 1+2: LN_in + transpose + dwn conv ============
            PD = DX_R * DX_W          # dx_dram partition stride (elements)
            SH_P = 11584              # sh_dram per-partition plane size
            IM2COL_SCHED = {}
            for t in range(9):
                ky = t // 3
                for T in range(3):
                    last_row = 16 + (ky - 1) * 16 + 24 * T + 24   # worst-case branch d=16
                    if ky == 0:
                        last_row = 24 * T + 24 + 16               # d=1..16: max rs0 = 15
                    g_ready = min((min(last_row, 104) + 7) // 8 - 1, SR // 8 - 1)
                    IM2COL_SCHED.setdefault(max(0, g_ready), []).append((t, T))
            for g in range(SR // 8):
                r0 = 8 * g
                xg = wk3.tile([128, 8, 256], F32, tag="xg")
                nc.gpsimd.dma_start(xg[:], x_in.ap()[r0:r0 + 8].rearrange("r w c -> w r c"))
                bst = st.tile([128, 8, 6], F32, tag="bst")
                for j in range(8):
                    nc.vector.bn_stats(bst[:, j, :], xg[:, j, :])
                mv = st.tile([128, 8, 2], F32, tag="mv")
                for j in range(8):
                    nc.vector.bn_aggr(mv[:, j, :], bst[:, j, :])
                sd = st.tile([128, 8], F32, tag="sd")
                nc.vector.tensor_scalar_add(sd[:], mv[:, :, 1], EPS)
                r = st.tile([128, 8], F32, tag="r")
                nc.scalar.activation(r[:], sd[:], AF.Abs_reciprocal_sqrt)

                nmr = st.tile([128, 8], F32, tag="nmr")
                nc.vector.tensor_tensor(nmr[:], mv[:, :, 0], r[:], op=ALU.mult)
                nc.vector.tensor_scalar_mul(nmr[:], nmr[:], -1.0)
                xnb = wk3.tile([128, 8, 256], BF16, tag="xnb")
                for j in range(8):
                    nc.scalar.activation(xnb[:, j, :], xg[:, j, :], AF.Identity,
                                         bias=nmr[:, j:j + 1], scale=r[:, j:j + 1])
                # transpose to channel-major (xt covers shard rows 16..88, 8-aligned)
                in_cat = 16 <= r0 < 88
                if in_cat:
                    dst = xt
                    roff = r0 - 16
                else:
                    dst = wk1.tile([128, 2, 8, XT_W], BF16, tag="xnt")
                    roff = 0
                for j in range(8):
                    tp = acc.tile([128, 256], BF16, tag="acc")
                    for ch in range(2):
                        nc.tensor.transpose(tp[:, 128 * ch:128 * ch + 128],
                                            xnb[:, j, 128 * ch:128 * ch + 128], Ident[:])
                    ev = nc.vector.tensor_copy if j % 2 == 0 else nc.scalar.copy
                    ev(dst[:, 0:2, roff + j, 1:129],
                       tp[:].rearrange("p (c b) -> p c b", c=2))
                # dwn conv on these 8 rows (two 4-row psums)
                dxs = wk3.tile([128, 8, 160], BF16, tag="dxs")
                for half in range(2):
                    pd = ps.tile([128, 512], F32, tag="mm")
                    pdv = pd[:].rearrange("p (a b) -> p a b", a=4)
                    for ch in range(2):
                        nc.tensor.matmul(pdv, Wdwn[:, ch, :],
                                         dst[:, ch, roff + 4 * half:roff + 4 * half + 4, 1:129],
                                         start=(ch == 0), stop=(ch == 1))
                    nc.scalar.activation(dxs[:, 4 * half:4 * half + 4, 16:144], pdv,
                                         AF.Relu, bias=Bdwn)
                nc.vector.memset(dxs[:, :, 0:16], 0.0)
                nc.vector.memset(dxs[:, :, 144:160], 0.0)
                nc.sync.dma_start(dx_dram[:, r0:r0 + 8, :], dxs[:])
                # im2col row-third copies whose source rows just completed
                for (t, T) in IM2COL_SCHED.get(g, []):
                    ky, kx = t // 3, t % 3
                    srcf = bass.AP(dx_dram[:].tensor,
                                   (16 + (ky - 1) + 24 * T) * DX_W,
                                   [[8 * PD + (ky - 1) * DX_W, 16], [PD, 8],
                                    [1, 24 * 160]])
                    dstf = bass.AP(sh_dram[:].tensor,
                                   t * 128 * SH_P + 32 - (kx - 1) + 24 * T * 160,
                                   [[8 * SH_P - (kx - 1), 16], [SH_P, 8],
                                    [1, 24 * 160]])
                    nc.sync.dma_start(dstf, srcf)

            # (im2col copies are interleaved into the phase-1/2 loop as row-thirds)

            # (im2col writes were interleaved into the phase-1/2 loop above)

            # ============ phase 4: k/q/v convs + exp(kq) ============
            PARTIALS = []
            psums_t = st.tile([128, 20], F32, tag="psums", name="psums_t")
            for c9 in range(9):
                j0 = 8 * c9
                grps = [g for g in range(2) if j0 + 4 * g < KR]
                pk, pq, pv = {}, {}, {}
                for g in grps:
                    pk[g] = acc.tile([128, 512], F32, tag="acc", name=f"pk{c9}_{g}")
                    pq[g] = acc.tile([128, 512], F32, tag="acc", name=f"pq{c9}_{g}")
                    pv[g] = acc.tile([128, 512], F32, tag="acc", name=f"pv{c9}_{g}")
                for tg in range(3):
                    sht = shp.tile([128, 3, 8, 160], BF16, tag="sht")
                    eng = nc.sync if tg % 2 == 0 else nc.scalar
                    eng.dma_start(
                        sht[:].rearrange("p t a b -> p t (a b)"),
                        sh_dram[3 * tg:3 * tg + 3, :,
                                32 + j0 * 160:32 + (j0 + 8) * 160].rearrange(
                                    "t p f -> p t f"))
                    for tt in range(3):
                        t = 3 * tg + tt
                        for grp in grps:
                            rhs = sht[:, tt, 4 * grp:4 * grp + 4, 16:144]
                            nc.tensor.matmul(pk[grp][:], Wk[:, t, :], rhs,
                                             start=(t == 0), stop=(t == 8))
                            nc.tensor.matmul(pq[grp][:], Wq[:, t, :], rhs,
                                             start=(t == 0), stop=(t == 8))
                            nc.tensor.matmul(pv[grp][:], Wv[:, t, :], rhs,
                                             start=(t == 0), stop=(t == 8))
                for grp in grps:
                    jb = j0 + 4 * grp
                    nrows = min(4, KR - jb)
                    nn = 128 * nrows
                    kev = wk.tile([128, 4, 128], BF16, tag="kev")
                    qev = wk.tile([128, 4, 128], BF16, tag="qev")
                    nc.scalar.activation(kev[:, 0:nrows, :], pk[grp][:, 0:nn], AF.Relu,
                                         bias=Bk)
                    nc.scalar.activation(qev[:, 0:nrows, :], pq[grp][:, 0:nn], AF.Relu,
                                         bias=Bq)
                    nc.scalar.activation(vbuf[:, jb:jb + nrows, :], pv[grp][:, 0:nn],
                                         AF.Relu, bias=Bv)
                    kqf = wk.tile([128, 4, 128], F32, tag="fscratch")
                    nc.vector.tensor_tensor(kqf[:, 0:nrows, :], kev[:, 0:nrows, :],
                                            qev[:, 0:nrows, :], op=ALU.mult)
                    nc.scalar.activation(ekq[:, jb:jb + nrows, :], kqf[:, 0:nrows, :],
                                         AF.Exp)
                    # partial softmax sum over the fresh rows restricted to [1, 65)
                    lo = max(jb, 1)
                    hi = min(jb + nrows, 65)
                    if lo < hi:
                        pidx = len(PARTIALS)
                        psl = psums_t[:, pidx:pidx + 1]
                        nc.vector.tensor_reduce(psl, ekq[:, lo:hi, :],
                                                axis=mybir.AxisListType.XY, op=ALU.add)
                        PARTIALS.append(pidx)

            # ============ phase 5: softmax sums + pairwise AllReduce ============
            sums = st.tile([128, 1], F32, tag="sums")
            nc.vector.tensor_reduce(sums[:], psums_t[:, 0:len(PARTIALS)],
                                    axis=mybir.AxisListType.X, op=ALU.add)
            nc.sync.dma_start(cc_in[:], sums[:])
            nc.gpsimd.collective_compute(
                "AllReduce", ALU.add,
                replica_groups=[[0, 1], [2, 3], [4, 5], [6, 7]],
                ins=[cc_in.opt()], outs=[cc_out.opt()],
            )
            tsum = st.tile([128, 1], F32, tag="tsum")
            nc.sync.dma_start(tsum[:], cc_out[:])
            rs = st.tile([128, 1], F32, tag="rs")
            nc.vector.reciprocal(rs[:], tsum[:])

            # ============ phase 6: attn*v + LN_up -> ybuf ============
            for c22 in range(22):
                j0 = 3 * c22
                avf = wk.tile([128, 3, 128], F32, tag="fscratch")
                nc.vector.tensor_tensor(avf[:], ekq[:, j0:j0 + 3, :],
                                        vbuf[:, j0:j0 + 3, :], op=ALU.mult)
                avb = wk.tile([128, 3, 128], BF16, tag="avb")
                nc.scalar.activation(avb[:], avf[:], AF.Identity, scale=rs[:])
                sqb = wk.tile([128, 3, 128], BF16, tag="sqb")
                nc.scalar.activation(sqb[:], avb[:], AF.Square)
                pm = acc.tile([128, 384], F32, tag="acc")
                pq2 = acc.tile([128, 384], F32, tag="acc")
                nc.tensor.matmul(pm[:], Gm[:], avb[:].rearrange("p a b -> p (a b)"),
                                 start=True, stop=True)
                nc.tensor.matmul(pq2[:], Gm[:], sqb[:].rearrange("p a b -> p (a b)"),
                                 start=True, stop=True)
                msb = wk1.tile([128, 384], F32, tag="msb")
                nc.vector.tensor_copy(msb[:], pm[:])
                m2 = wk1.tile([128, 384], F32, tag="m2")
                nc.scalar.activation(m2[:], msb[:], AF.Square)
                varu = wk1.tile([128, 384], F32, tag="varu")
                nc.vector.tensor_tensor(varu[:], pq2[:], m2[:], op=ALU.subtract)
                nc.vector.tensor_scalar_add(varu[:], varu[:], EPS)
                ru = wk1.tile([128, 384], F32, tag="ru")
                nc.scalar.activation(ru[:], varu[:], AF.Abs_reciprocal_sqrt)
                yt = wk1.tile([128, 384], F32, tag="yt")
                nc.vector.tensor_tensor(yt[:], avb[:].rearrange("p a b -> p (a b)"),
                                        msb[:], op=ALU.subtract)
                nc.vector.tensor_tensor(
                    ybuf[:, j0:j0 + 3, 1:129],
                    yt[:].rearrange("p (a b) -> p a b", a=3),
                    ru[:].rearrange("p (a b) -> p a b", a=3), op=ALU.mult)

            # ============ phase 7+8: smooth conv + LN_out ============
            for pc in range(16):
                r0 = 4 * pc
                op_t = wk.tile([128, 2, 4, 128], BF16, tag="opt")
                for mc in range(2):
                    psm = ps.tile([128, 512], F32, tag="mm")
                    first = True
                    for t in range(9):
                        ky, kx = t // 3, t % 3
                        for kc in range(3):
                            if kc < 2:
                                rhs = xt[:, kc, r0 + ky:r0 + ky + 4, kx:kx + 128]
                            else:
                                rhs = ybuf[:, r0 + ky:r0 + ky + 4, kx:kx + 128]
                            nc.tensor.matmul(
                                psm[:], Ws[:, t, kc, 128 * mc:128 * mc + 128], rhs,
                                start=first, stop=(t == 8 and kc == 2))
                            first = False
                    nc.scalar.activation(op_t[:, mc, :, :],
                                         psm[:].rearrange("p (a b) -> p a b", a=4),
                                         AF.Relu, bias=Bias[:, 4 + mc:5 + mc])
                # transpose to pixel-major
                on_t = wk.tile([128, 4, 256], BF16, tag="ont")
                for j in range(4):
                    for mc in range(2):
                        tp = acc.tile([128, 256], BF16, tag="acc")
                        nc.tensor.transpose(tp[:, 0:128], op_t[:, mc, j, :], Ident[:])
                        nc.vector.tensor_copy(on_t[:, j, 128 * mc:128 * mc + 128],
                                              tp[:, 0:128])
                # LN_out (over 256 channels, free dim now)
                bsto = st.tile([128, 4, 6], F32, tag="bsto")
                for j in range(4):
                    nc.vector.bn_stats(bsto[:, j, :], on_t[:, j, :])
                mvo = st.tile([128, 4, 2], F32, tag="mvo")
                for j in range(4):
                    nc.vector.bn_aggr(mvo[:, j, :], bsto[:, j, :])
                sdo = st.tile([128, 4], F32, tag="sdo")
                nc.vector.tensor_scalar_add(sdo[:], mvo[:, :, 1], EPS)
                ro = st.tile([128, 4], F32, tag="ro")
                nc.scalar.activation(ro[:], sdo[:], AF.Abs_reciprocal_sqrt)
                orow = wk3.tile([128, 4, 256], F32, tag="xg")
                for j in range(4):
                    nc.vector.tensor_scalar(orow[:, j, :], on_t[:, j, :],
                                            mvo[:, j, 0:1], ro[:, j:j + 1],
                                            op0=ALU.subtract, op1=ALU.mult)
                nc.gpsimd.dma_start(out.ap()[r0:r0 + 4].rearrange("r w c -> w r c"),
                                     orow[:])

    nc.compile()
    return nc


# --------------------------------------------------------------------------
# public entry point
# --------------------------------------------------------------------------
def kernel(**inputs):
    if "nc" not in _CACHE:
        _CACHE["nc"] = build_kernel()
    nc = _CACHE["nc"]

    wts = _prep_weights(inputs)
    shards = _shard_x(np.asarray(inputs["x"], np.float32))
    in_maps = []
    for c in range(N_CORES):
        m = {"x": shards[c]}
        m.update(wts)
        in_maps.append(m)
    res = run_bass_kernel_spmd(nc, in_maps, core_ids=list(range(N_CORES)))
    full = np.empty((N, H, W, C), np.float32)
    for c in range(N_CORES):
        n, v = c // 2, c % 2
        full[n, 64 * v:64 * v + 64] = res.results[c]["out"]
    return full


# revision 24
# speedup vs baseline: 2.4907x; 1.0316x over previous
"""Trainium2 Bass kernel for nn_AtrousAttentionBlock (16 dilated attention branches + smooth conv).

Sharding: 8 cores = (image n in [0,4)) x (vertical half v in [0,2)); core 2n+v computes
output rows [64v, 64v+64) of image n from a zero-padded 100-row x shard (17-row halos).
On-chip layout is channel-major bf16 (f32 PSUM accumulation):
  phase 1: LN_in (pixel-major stats) + PE transpose -> xn channel-major
  phase 2: 1x1 "dwn" conv, all 16 branches packed as 128 partitions (16br x 8ch)
  phase 3: per-branch dilation shifts materialized via DRAM im2col round trip
  phase 4: k/q/v 3x3 dilated convs as block-diagonal-weight matmuls (9 taps accumulated)
  phase 5: spatial-softmax partial sums + pairwise AllReduce (image halves)
  phase 6: attn*v + LN_up (stats via block-ones matmul, broadcast fused)
  phase 7: 3x3 smooth conv 384->256 (27 accumulating K-chunk matmuls per pixel tile)
  phase 8: LN_out in pixel-major after PE transpose, DMA out in NHWC
"""
import numpy as np
import ml_dtypes

import concourse.bass as bass
import concourse.bacc as bacc
import concourse.tile as tile
import concourse.mybir as mybir
from concourse.bass_utils import run_bass_kernel_spmd

N, H, W, C = 4, 128, 128, 256
ND, DF = 16, 8
CAT = C + ND * DF
EPS = 1e-3
N_CORES = 8

HALO = 17        # x halo rows per side (16 for dilation-16 taps + 1 for smooth conv)
SR = 104         # stored shard rows (98 used, padded to 8-row groups)
XT_W = 130       # padded width of channel-major xn/y (1 zero col per side)
XT_R = 72        # xn rows kept for the smooth conv (shard rows 16..88, 8-row aligned)
DX_R = 104       # dx rows ([0,98) real + zero tail so all 9x8-row kqv chunks stay in range)
DX_W = 160       # dx padded width (16 zero cols per side)
KR = 66          # kqv rows used: image rows 64v-1 .. 64v+65

F32 = mybir.dt.float32
BF16 = mybir.dt.bfloat16
AF = mybir.ActivationFunctionType
ALU = mybir.AluOpType

_CACHE = {}


# --------------------------------------------------------------------------
# host-side preparation
# --------------------------------------------------------------------------
def _prep_weights(inp):
    """Fold LN affines into conv weights; build PE-layout weight matrices."""
    for k in ("ln_in_b", "ln_up_b", "ln_out_b"):
        assert np.all(np.asarray(inp[k]) == 0), f"{k} != 0 unsupported"
    assert np.all(np.asarray(inp["ln_out_g"]) == 1), "ln_out_g != 1 unsupported"
    g_in = np.asarray(inp["ln_in_g"], np.float64)
    g_up = np.asarray(inp["ln_up_g"], np.float64)

    W_dwn = np.zeros((C, 128), np.float64)
    for i in range(ND):
        W_dwn[:, 8 * i:8 * i + 8] = inp["dwn_w"][i, 0, 0] * g_in[:, None]

    def blockdiag(w):  # [ND,3,3,DF,DF] -> [9,128,128]
        out = np.zeros((9, 128, 128), np.float64)
        for t in range(9):
            ky, kx = t // 3, t % 3
            for i in range(ND):
                out[t, 8 * i:8 * i + 8, 8 * i:8 * i + 8] = w[i, ky, kx]
        return out

    Ws = np.asarray(inp["smooth_w"], np.float64).copy()
    Ws[:, :, :C, :] *= g_in[None, None, :, None]
    for i in range(ND):
        Ws[:, :, C + 8 * i:C + 8 * i + 8, :] *= g_up[None, None, :, None]

    Gm = np.zeros((128, 128), np.float64)   # per-branch mean matrix (ones8x8 / 8)
    for i in range(ND):
        Gm[8 * i:8 * i + 8, 8 * i:8 * i + 8] = 1.0 / 8.0

    bf = lambda a: np.ascontiguousarray(np.asarray(a, np.float32)).astype(ml_dtypes.bfloat16)
    f32c = lambda a: np.ascontiguousarray(np.asarray(a, np.float32))
    biases = np.stack([
        np.asarray(inp["dwn_b"]).reshape(128),
        np.asarray(inp["k_b"]).reshape(128),
        np.asarray(inp["q_b"]).reshape(128),
        np.asarray(inp["v_b"]).reshape(128),
        np.asarray(inp["smooth_b"])[0:128],
        np.asarray(inp["smooth_b"])[128:256],
    ], axis=1)
    return dict(
        wdwn=bf(W_dwn),
        wk=bf(blockdiag(inp["k_w"])), wq=bf(blockdiag(inp["q_w"])), wv=bf(blockdiag(inp["v_w"])),
        ws=bf(Ws.reshape(9, 3, 128, 256)),
        biases=f32c(biases),
        gm=bf(Gm),
        ident=bf(np.eye(128)),
    )


def _shard_x(x):
    shards = []
    for c in range(N_CORES):
        n, v = c // 2, c % 2
        sh = np.zeros((SR, W, C), np.float32)
        lo, hi = 64 * v - HALO, 64 * v + 81
        slo, shi = max(0, lo), min(H, hi)
        sh[slo - lo:shi - lo] = x[n, slo:shi]
        shards.append(np.ascontiguousarray(sh))
    return shards


# --------------------------------------------------------------------------
# device kernel
# --------------------------------------------------------------------------
def build_kernel():
    nc = bacc.Bacc("TRN2", target_bir_lowering=False, debug=False, num_devices=N_CORES)

    x_in = nc.dram_tensor("x", [SR, W, C], F32, kind="ExternalInput")
    wdwn_in = nc.dram_tensor("wdwn", [C, 128], BF16, kind="ExternalInput")
    wk_in = nc.dram_tensor("wk", [9, 128, 128], BF16, kind="ExternalInput")
    wq_in = nc.dram_tensor("wq", [9, 128, 128], BF16, kind="ExternalInput")
    wv_in = nc.dram_tensor("wv", [9, 128, 128], BF16, kind="ExternalInput")
    ws_in = nc.dram_tensor("ws", [9, 3, 128, 256], BF16, kind="ExternalInput")
    bias_in = nc.dram_tensor("biases", [128, 6], F32, kind="ExternalInput")
    gm_in = nc.dram_tensor("gm", [128, 128], BF16, kind="ExternalInput")
    id_in = nc.dram_tensor("ident", [128, 128], BF16, kind="ExternalInput")
    out = nc.dram_tensor("out", [64, W, C], F32, kind="ExternalOutput")

    with tile.TileContext(nc) as tc:
        with (
            tc.tile_pool(name="const", bufs=1) as cp,
            tc.tile_pool(name="big", bufs=1) as bp,
            tc.tile_pool(name="work", bufs=2) as wk,
            tc.tile_pool(name="wk3", bufs=3) as wk3,
            tc.tile_pool(name="wk1", bufs=1) as wk1,
            tc.tile_pool(name="sht", bufs=2) as shp,
            tc.tile_pool(name="st", bufs=3) as st,
            tc.tile_pool(name="ps", bufs=2, space="PSUM") as ps,
            tc.tile_pool(name="acc", bufs=6, space="PSUM") as acc,
            tc.tile_pool(name="dram", bufs=1, space="DRAM") as dram,
        ):
            # ---- constants ----
            Wdwn = cp.tile([128, 2, 128], BF16, tag="wdwn")
            nc.sync.dma_start(Wdwn[:], wdwn_in.ap().rearrange("(c p) m -> p c m", p=128))
            Wk = cp.tile([128, 9, 128], BF16, tag="wk")
            Wq = cp.tile([128, 9, 128], BF16, tag="wq")
            Wv = cp.tile([128, 9, 128], BF16, tag="wv")
            Ws = cp.tile([128, 9, 3, 256], BF16, tag="ws")
            nc.sync.dma_start(Wk[:], wk_in.ap().rearrange("t p m -> p t m"))
            nc.sync.dma_start(Wq[:], wq_in.ap().rearrange("t p m -> p t m"))
            nc.sync.dma_start(Wv[:], wv_in.ap().rearrange("t p m -> p t m"))
            nc.sync.dma_start(Ws[:], ws_in.ap().rearrange("t k p m -> p t k m"))
            Bias = cp.tile([128, 6], F32, tag="bias")
            nc.sync.dma_start(Bias[:], bias_in.ap())
            Bdwn = Bias[:, 0:1]
            Bk = Bias[:, 1:2]
            Bq = Bias[:, 2:3]
            Bv = Bias[:, 3:4]
            Gm = cp.tile([128, 128], BF16, tag="gm")
            Ident = cp.tile([128, 128], BF16, tag="id")
            nc.sync.dma_start(Gm[:], gm_in.ap())
            nc.sync.dma_start(Ident[:], id_in.ap())

            # ---- persistent activations ----
            xt = bp.tile([128, 2, XT_R, XT_W], BF16, tag="xt")    # xn shard rows 16..84
            ekq = bp.tile([128, KR, 128], BF16, tag="ekq")
            vbuf = bp.tile([128, KR, 128], BF16, tag="vbuf")
            ybuf = bp.tile([128, KR, XT_W], BF16, tag="ybuf")

            nc.vector.memset(xt[:, :, :, 0:1], 0.0)
            nc.vector.memset(xt[:, :, :, XT_W - 1:XT_W], 0.0)
            nc.vector.memset(ybuf[:, :, 0:1], 0.0)
            nc.vector.memset(ybuf[:, :, XT_W - 1:XT_W], 0.0)

            # dx lives in DRAM; per-branch tap shifts are linear in branch index so
            # the whole im2col is 9 arithmetic-stride DRAM->DRAM copies.
            dx_dram = dram.tile([128, DX_R, DX_W], BF16, tag="dxd")
            sh_dram = dram.tile([9, 128, 32 + 72 * 160 + 32], BF16, tag="shd")
            # zero-fill the 16-col side pads of dx_dram once
            zpad = wk1.tile([128, 104, 16], BF16, tag="zpad")
            nc.vector.memset(zpad[:], 0.0)
            nc.sync.dma_start(dx_dram[:, :, 0:16], zpad[:])
            nc.sync.dma_start(dx_dram[:, :, DX_W - 16:DX_W], zpad[:])
            cc_in = dram.tile([128, 1], F32, tag="ccin")
            cc_out = dram.tile([128, 1], F32, tag="ccout")

            # ============ phase 1+2: LN_in + transpose + dwn conv ============
            PD = DX_R * DX_W          # dx_dram partition stride (elements)
            SH_P = 11584              # sh_dram per-partition plane size
            THIRDS = [(0, 24), (24, 48), (48, 66)]
            IM2COL_SCHED = {}
            for t in range(9):
                ky = t // 3
                for T, (ra, rb) in enumerate(THIRDS):
                    last_row = (15 if ky == 0 else 16 + (ky - 1) * 16) + rb
                    g_ready = min((min(last_row, 104) + 7) // 8 - 1, SR // 8 - 1)
                    IM2COL_SCHED.setdefault(max(0, g_ready), []).append((t, T))
            for g in range(SR // 8):
                r0 = 8 * g
                xg = wk3.tile([128, 8, 256], F32, tag="xg")
                nc.gpsimd.dma_start(xg[:], x_in.ap()[r0:r0 + 8].rearrange("r w c -> w r c"))
                bst = st.tile([128, 8, 6], F32, tag="bst")
                for j in range(8):
                    nc.vector.bn_stats(bst[:, j, :], xg[:, j, :])
                mv = st.tile([128, 8, 2], F32, tag="mv")
                for j in range(8):
                    nc.vector.bn_aggr(mv[:, j, :], bst[:, j, :])
                sd = st.tile([128, 8], F32, tag="sd")
                nc.vector.tensor_scalar_add(sd[:], mv[:, :, 1], EPS)
                r = st.tile([128, 8], F32, tag="r")
                nc.scalar.activation(r[:], sd[:], AF.Abs_reciprocal_sqrt)

                xnb = wk3.tile([128, 8, 256], BF16, tag="xnb")
                for j in range(8):
                    nc.vector.tensor_scalar(xnb[:, j, :], xg[:, j, :],
                                            mv[:, j, 0:1], r[:, j:j + 1],
                                            op0=ALU.subtract, op1=ALU.mult)
                # transpose to channel-major (xt covers shard rows 16..88, 8-aligned)
                in_cat = 16 <= r0 < 88
                if in_cat:
                    dst = xt
                    roff = r0 - 16
                else:
                    dst = wk1.tile([128, 2, 8, XT_W], BF16, tag="xnt")
                    roff = 0
                for j in range(8):
                    tp = acc.tile([128, 256], BF16, tag="acc")
                    for ch in range(2):
                        nc.tensor.transpose(tp[:, 128 * ch:128 * ch + 128],
                                            xnb[:, j, 128 * ch:128 * ch + 128], Ident[:])
                    nc.scalar.copy(dst[:, 0:2, roff + j, 1:129],
                                   tp[:].rearrange("p (c b) -> p c b", c=2))
                # dwn conv on these 8 rows (two 4-row psums)
                dxs = wk3.tile([128, 8, 160], BF16, tag="dxs")
                for half in range(2):
                    pd = ps.tile([128, 512], F32, tag="mm")
                    pdv = pd[:].rearrange("p (a b) -> p a b", a=4)
                    for ch in range(2):
                        nc.tensor.matmul(pdv, Wdwn[:, ch, :],
                                         dst[:, ch, roff + 4 * half:roff + 4 * half + 4, 1:129],
                                         start=(ch == 0), stop=(ch == 1))
                    nc.scalar.activation(dxs[:, 4 * half:4 * half + 4, 16:144], pdv,
                                         AF.Relu, bias=Bdwn)
                nc.vector.memset(dxs[:, :, 0:16], 0.0)
                nc.vector.memset(dxs[:, :, 144:160], 0.0)
                nc.sync.dma_start(dx_dram[:, r0:r0 + 8, :], dxs[:])
                # im2col row-third copies whose source rows just completed
                for (t, T) in IM2COL_SCHED.get(g, []):
                    ky, kx = t // 3, t % 3
                    ra, rb = THIRDS[T]
                    nrow = rb - ra
                    srcf = bass.AP(dx_dram[:].tensor,
                                   (16 + (ky - 1) + ra) * DX_W,
                                   [[8 * PD + (ky - 1) * DX_W, 16], [PD, 8],
                                    [1, nrow * 160]])
                    dstf = bass.AP(sh_dram[:].tensor,
                                   t * 128 * SH_P + 32 - (kx - 1) + ra * 160,
                                   [[8 * SH_P - (kx - 1), 16], [SH_P, 8],
                                    [1, nrow * 160]])
                    nc.sync.dma_start(dstf, srcf)

            # (im2col copies are interleaved into the phase-1/2 loop as row-thirds)

            # (im2col writes were interleaved into the phase-1/2 loop above)

            # ============ phase 4: k/q/v convs + exp(kq) ============
            PARTIALS = []
            psums_t = st.tile([128, 20], F32, tag="psums", name="psums_t")
            for c9 in range(9):
                j0 = 8 * c9
                grps = [g for g in range(2) if j0 + 4 * g < KR]
                pk, pq, pv = {}, {}, {}
                for g in grps:
                    pk[g] = acc.tile([128, 512], F32, tag="acc", name=f"pk{c9}_{g}")
                    pq[g] = acc.tile([128, 512], F32, tag="acc", name=f"pq{c9}_{g}")
                    pv[g] = acc.tile([128, 512], F32, tag="acc", name=f"pv{c9}_{g}")
                for tg in range(3):
                    sht = shp.tile([128, 3, 8, 160], BF16, tag="sht")
                    eng = nc.sync if tg % 2 == 0 else nc.scalar
                    eng.dma_start(
                        sht[:].rearrange("p t a b -> p t (a b)"),
                        sh_dram[3 * tg:3 * tg + 3, :,
                                32 + j0 * 160:32 + (j0 + 8) * 160].rearrange(
                                    "t p f -> p t f"))
                    for tt in range(3):
                        t = 3 * tg + tt
                        for grp in grps:
                            nr = min(4, KR - (j0 + 4 * grp))
                            rhs = sht[:, tt, 4 * grp:4 * grp + nr, 16:144]
                            nn = 128 * nr
                            nc.tensor.matmul(pk[grp][:, 0:nn], Wk[:, t, :], rhs,
                                             start=(t == 0), stop=(t == 8))
                            nc.tensor.matmul(pq[grp][:, 0:nn], Wq[:, t, :], rhs,
                                             start=(t == 0), stop=(t == 8))
                            nc.tensor.matmul(pv[grp][:, 0:nn], Wv[:, t, :], rhs,
                                             start=(t == 0), stop=(t == 8))
                for grp in grps:
                    jb = j0 + 4 * grp
                    nrows = min(4, KR - jb)
                    nn = 128 * nrows
                    kev = wk.tile([128, 4, 128], BF16, tag="kev")
                    qev = wk.tile([128, 4, 128], BF16, tag="qev")
                    nc.scalar.activation(kev[:, 0:nrows, :], pk[grp][:, 0:nn], AF.Relu,
                                         bias=Bk)
                    nc.scalar.activation(qev[:, 0:nrows, :], pq[grp][:, 0:nn], AF.Relu,
                                         bias=Bq)
                    nc.scalar.activation(vbuf[:, jb:jb + nrows, :], pv[grp][:, 0:nn],
                                         AF.Relu, bias=Bv)
                    kqf = wk.tile([128, 4, 128], F32, tag="fscratch")
                    nc.vector.tensor_tensor(kqf[:, 0:nrows, :], kev[:, 0:nrows, :],
                                            qev[:, 0:nrows, :], op=ALU.mult)
                    nc.scalar.activation(ekq[:, jb:jb + nrows, :], kqf[:, 0:nrows, :],
                                         AF.Exp)
                    # partial softmax sum over the fresh rows restricted to [1, 65)
                    lo = max(jb, 1)
                    hi = min(jb + nrows, 65)
                    if lo < hi:
                        pidx = len(PARTIALS)
                        psl = psums_t[:, pidx:pidx + 1]
                        nc.vector.tensor_reduce(psl, ekq[:, lo:hi, :],
                                                axis=mybir.AxisListType.XY, op=ALU.add)
                        PARTIALS.append(pidx)

            # ============ phase 5: softmax sums + pairwise AllReduce ============
            sums = st.tile([128, 1], F32, tag="sums")
            nc.vector.tensor_reduce(sums[:], psums_t[:, 0:len(PARTIALS)],
                                    axis=mybir.AxisListType.X, op=ALU.add)
            nc.sync.dma_start(cc_in[:], sums[:])
            nc.gpsimd.collective_compute(
                "AllReduce", ALU.add,
                replica_groups=[[0, 1], [2, 3], [4, 5], [6, 7]],
                ins=[cc_in.opt()], outs=[cc_out.opt()],
            )
            tsum = st.tile([128, 1], F32, tag="tsum")
            nc.sync.dma_start(tsum[:], cc_out[:])
            rs = st.tile([128, 1], F32, tag="rs")
            nc.vector.reciprocal(rs[:], tsum[:])

            # ============ phase 6: attn*v + LN_up -> ybuf ============
            for c22 in range(22):
                j0 = 3 * c22
                avf = wk.tile([128, 3, 128], F32, tag="fscratch")
                nc.vector.tensor_tensor(avf[:], ekq[:, j0:j0 + 3, :],
                                        vbuf[:, j0:j0 + 3, :], op=ALU.mult)
                avb = wk.tile([128, 3, 128], BF16, tag="avb")
                nc.scalar.activation(avb[:], avf[:], AF.Identity, scale=rs[:])
                sqb = wk.tile([128, 3, 128], BF16, tag="sqb")
                nc.scalar.activation(sqb[:], avb[:], AF.Square)
                pm = acc.tile([128, 384], F32, tag="acc")
                pq2 = acc.tile([128, 384], F32, tag="acc")
                nc.tensor.matmul(pm[:], Gm[:], avb[:].rearrange("p a b -> p (a b)"),
                                 start=True, stop=True)
                nc.tensor.matmul(pq2[:], Gm[:], sqb[:].rearrange("p a b -> p (a b)"),
                                 start=True, stop=True)
                msb = wk1.tile([128, 384], F32, tag="msb")
                nc.vector.tensor_copy(msb[:], pm[:])
                m2 = wk1.tile([128, 384], F32, tag="m2")
                nc.scalar.activation(m2[:], msb[:], AF.Square)
                varu = wk1.tile([128, 384], F32, tag="varu")
                nc.vector.tensor_tensor(varu[:], pq2[:], m2[:], op=ALU.subtract)
                nc.vector.tensor_scalar_add(varu[:], varu[:], EPS)
                ru = wk1.tile([128, 384], F32, tag="ru")
                nc.scalar.activation(ru[:], varu[:], AF.Abs_reciprocal_sqrt)
                yt = wk1.tile([128, 384], F32, tag="yt")
                nc.vector.tensor_tensor(yt[:], avb[:].rearrange("p a b -> p (a b)"),
                                        msb[:], op=ALU.subtract)
                nc.vector.tensor_tensor(
                    ybuf[:, j0:j0 + 3, 1:129],
                    yt[:].rearrange("p (a b) -> p a b", a=3),
                    ru[:].rearrange("p (a b) -> p a b", a=3), op=ALU.mult)

            # ============ phase 7+8: smooth conv + LN_out ============
            for pc in range(16):
                r0 = 4 * pc
                op_t = wk.tile([128, 2, 4, 128], BF16, tag="opt")
                for mc in range(2):
                    psm = ps.tile([128, 512], F32, tag="mm")
                    first = True
                    for t in range(9):
                        ky, kx = t // 3, t % 3
                        for kc in range(3):
                            if kc < 2:
                                rhs = xt[:, kc, r0 + ky:r0 + ky + 4, kx:kx + 128]
                            else:
                                rhs = ybuf[:, r0 + ky:r0 + ky + 4, kx:kx + 128]
                            nc.tensor.matmul(
                                psm[:], Ws[:, t, kc, 128 * mc:128 * mc + 128], rhs,
                                start=first, stop=(t == 8 and kc == 2))
                            first = False
                    nc.scalar.activation(op_t[:, mc, :, :],
                                         psm[:].rearrange("p (a b) -> p a b", a=4),
                                         AF.Relu, bias=Bias[:, 4 + mc:5 + mc])
                # transpose to pixel-major
                on_t = wk.tile([128, 4, 256], BF16, tag="ont")
                for j in range(4):
                    for mc in range(2):
                        tp = acc.tile([128, 256], BF16, tag="acc")
                        nc.tensor.transpose(tp[:, 0:128], op_t[:, mc, j, :], Ident[:])
                        nc.vector.tensor_copy(on_t[:, j, 128 * mc:128 * mc + 128],
                                              tp[:, 0:128])
                # LN_out (over 256 channels, free dim now)
                bsto = st.tile([128, 4, 6], F32, tag="bsto")
                for j in range(4):
                    nc.vector.bn_stats(bsto[:, j, :], on_t[:, j, :])
                mvo = st.tile([128, 4, 2], F32, tag="mvo")
                for j in range(4):
                    nc.vector.bn_aggr(mvo[:, j, :], bsto[:, j, :])
                sdo = st.tile([128, 4], F32, tag="sdo")
                nc.vector.tensor_scalar_add(sdo[:], mvo[:, :, 1], EPS)
                ro = st.tile([128, 4], F32, tag="ro")
                nc.scalar.activation(ro[:], sdo[:], AF.Abs_reciprocal_sqrt)
                orow = wk3.tile([128, 4, 256], F32, tag="xg")
                for j in range(4):
                    nc.vector.tensor_scalar(orow[:, j, :], on_t[:, j, :],
                                            mvo[:, j, 0:1], ro[:, j:j + 1],
                                            op0=ALU.subtract, op1=ALU.mult)
                nc.gpsimd.dma_start(out.ap()[r0:r0 + 4].rearrange("r w c -> w r c"),
                                     orow[:])

    nc.compile()
    return nc


# --------------------------------------------------------------------------
# public entry point
# --------------------------------------------------------------------------
def kernel(**inputs):
    if "nc" not in _CACHE:
        _CACHE["nc"] = build_kernel()
    nc = _CACHE["nc"]

    wts = _prep_weights(inputs)
    shards = _shard_x(np.asarray(inputs["x"], np.float32))
    in_maps = []
    for c in range(N_CORES):
        m = {"x": shards[c]}
        m.update(wts)
        in_maps.append(m)
    res = run_bass_kernel_spmd(nc, in_maps, core_ids=list(range(N_CORES)))
    full = np.empty((N, H, W, C), np.float32)
    for c in range(N_CORES):
        n, v = c // 2, c % 2
        full[n, 64 * v:64 * v + 64] = res.results[c]["out"]
    return full
